# revision 31
# baseline (speedup 1.0000x reference)
"""BinarizedLeNet5/CIFAR10 Trainium2 kernel (8-core data parallel), v4.

The graded metric is wall-clock of a warm run_bass_kernel_spmd call, and
the axon tunnel moves ~50 MB/s — so v2's 287 MB of host-staged im2col
inputs WAS the runtime.  v4 ships ~16.5 MB instead:

- x goes up as int20 fixed-point over [-6,6): u16 hi + nibble-packed u4
  mantissa extension, unpadded, samples innermost.  The im2col expansion
  happens on-device with 36 strided gather DMAs per chunk-half (3-dim
  APs, contiguous (col,s) runs); halo positions keep a one-time q0
  memset that decodes to exact 0.  DVE unpacks the nibbles, decodes to
  f32 and splits into the bf16 hi/lo pair the conv1 matmuls consume
  (the split residual ~x*2^-18 dominates the encoding error; rel err
  1.12e-2 vs the 2e-2 gate).  conv1 psum free layout is (r, w, s).
- fc1w+fc2w ship bit-packed in one [128,2176] u8 tensor, unpacked
  on-device with DVE shift/and ops.
- all weight tensors arrive 8-way column-sharded and are AllGather'd
  on-device (replicated upload would cost 8x the bytes).
- small f32 constants consolidate into one [128,32] tensor, w1/w3h/w3l
  into one [128,168] bf16 tensor: 6 input args total.

Everything downstream of conv1's ACT-sign (sp/ic2 build, conv2, fc
phase, log_softmax tail) is byte-identical to v2.  A persistent XLA
compilation cache is configured at import so warm calls skip the
~0.5s client-side BIR re-verification that run_bass_kernel_spmd's
per-call re-jit otherwise pays.
"""
import sys
import numpy as np

sys.path.insert(0, "/opt/pypackages")
sys.path.insert(0, "/opt/trn_rl_repo")

import ml_dtypes

# Persistent XLA compilation cache: run_bass_kernel_spmd re-jits on every
# call (fresh closure), and without this each warm call pays ~0.5s of
# client-side BIR re-verification + DVE table gen before the NEFF cache
# hits.  The persistent cache keys on HLO fingerprint, so call 2+ skips
# backend compile entirely.
import jax

jax.config.update("jax_compilation_cache_dir", "/tmp/jax_comp_cache")
jax.config.update("jax_persistent_cache_min_entry_size_bytes", -1)
jax.config.update("jax_persistent_cache_min_compile_time_secs", 0.0)

BF = ml_dtypes.bfloat16
F8 = ml_dtypes.float8_e4m3
NCORES = 8
B = 2048
BC = B // NCORES          # 256 samples per core
CH = 16                   # samples per chunk
NCHUNK = BC // CH         # 16 chunks
EPS = np.float32(1e-5)
XCHW = 3 * 32 * 32 * CH   # chunk elements, unpadded (49152)
XCHM = XCHW // 2          # nibble-packed chunk bytes (24576)
XSC = float(2 ** 20) / 12.0   # int20 fixed-point scale over [-6, 6)
Q0 = 1 << 19                  # encoding of x == 0.0
C_HI = 3.0 * 2.0 ** -14       # decode: x = hi16*C_HI + m4*C_LO - 6
C_LO = 3.0 * 2.0 ** -18

_nc_cache = {}


def _f32(x):
    return np.asarray(x, np.float32)


def _host_prep(inputs):
    """Build all per-core device input arrays (layout prep only)."""
    x = _f32(inputs["x"])                      # [2048,3,32,32]

    # ---- x as int20 fixed-point over [-6,6): u16 hi + u4 nibble ext,
    # unpadded, samples innermost (halo positions stay at the q0 memset) ----
    q = np.clip(np.round((x.astype(np.float64) + 6.0) * XSC),
                0, 2 ** 20 - 1).astype(np.uint32)

    def stage(xq):
        xt = xq.reshape(NCORES, NCHUNK, CH, 3, 32, 32)
        xt = np.ascontiguousarray(xt.transpose(0, 1, 3, 4, 5, 2))
        return xt.reshape(NCORES, NCHUNK, -1)

    xh = stage((q >> 4).astype(np.uint16))
    m4 = stage((q & 15).astype(np.uint8))       # [.., pos, s]
    m4 = m4.reshape(NCORES, NCHUNK, XCHM, 2)    # byte = (s even, s odd)
    xl = (m4[..., 0] | (m4[..., 1] << 4)).astype(np.uint8)

    # ---- conv1 stationary: block-diag, k = 3*(3dy+dx)+c, out p = 4*co+j ----
    w1s = np.sign(_f32(inputs["conv1_w"]))               # [32,3,3,3]
    w1k = np.ascontiguousarray(w1s.transpose(2, 3, 1, 0)).reshape(27, 32)
    # bf16 blob: [:,0:128]=w1, [:,128:148]=w3h (kt-major), [:,148:168]=w3l
    wbf = np.zeros((128, 168), BF)
    for j in range(4):
        for co in range(32):
            wbf[32 * j:32 * j + 27, 4 * co + j] = w1k[:, co].astype(BF)
    w3 = _f32(inputs["fc3_w"]).T                         # [256,10]
    w3h = w3.astype(BF)
    w3l = (w3 - w3h.astype(np.float32)).astype(BF)
    for kt in range(2):
        wbf[:, 128 + 10 * kt:138 + 10 * kt] = w3h[128 * kt:128 * (kt + 1)]
        wbf[:, 148 + 10 * kt:158 + 10 * kt] = w3l[128 * kt:128 * (kt + 1)]

    # ---- conv2 stationaries [3][96,64] fp8: p = 32*dx + c ----
    w2s = np.sign(_f32(inputs["conv2_w"]))               # [64,32,3,3]
    w2_st = np.zeros((3, 96, 64), F8)
    for dy in range(3):
        for dx in range(3):
            w2_st[dy, 32 * dx:32 * dx + 32] = w2s[:, :, dy, dx].T.astype(F8)

    # ---- consolidated f32 constants [128, 32] (padded for 8-way shard) ----
    cst = np.zeros((128, 32), np.float32)
    inv1 = _f32(inputs["bn1_g"]) / np.sqrt(_f32(inputs["bn1_v"]) + EPS)
    sh1c = (_f32(inputs["conv1_b"]) - _f32(inputs["bn1_m"])) * inv1 \
        + _f32(inputs["bn1_b"])
    cst[:, 0] = np.repeat(inv1, 4)
    cst[:, 1] = np.repeat(sh1c, 4)
    inv2 = _f32(inputs["bn2_g"]) / np.sqrt(_f32(inputs["bn2_v"]) + EPS)
    sh2c = _f32(inputs["bn2_b"]) - _f32(inputs["bn2_m"]) * inv2
    cst[:, 2] = np.tile(inv2, 2)
    cst[:, 3] = np.tile(_f32(inputs["conv2_b"]) * inv2 + sh2c, 2)
    inv3 = _f32(inputs["bn3_g"]) / np.sqrt(_f32(inputs["bn3_v"]) + EPS)
    sh3c = (_f32(inputs["fc1_b"]) - _f32(inputs["bn3_m"])) * inv3 \
        + _f32(inputs["bn3_b"])
    cst[:, 4:8] = inv3.reshape(4, 128).T
    cst[:, 8:12] = sh3c.reshape(4, 128).T
    inv4 = _f32(inputs["bn4_g"]) / np.sqrt(_f32(inputs["bn4_v"]) + EPS)
    sh4c = (_f32(inputs["fc2_b"]) - _f32(inputs["bn4_m"])) * inv4 \
        + _f32(inputs["bn4_b"])
    cst[:, 12:14] = inv4.reshape(2, 128).T
    cst[:, 14:16] = sh4c.reshape(2, 128).T
    cst[:, 16:26] = _f32(inputs["fc3_b"])[None, :]

    # ---- fc1+fc2 bit-packed into one [128, 2176] u8 tensor ----
    # fc1: kt = yx//2, kp = c + 64*(yx%2), bits along m (cols 0:2048)
    fw1 = np.sign(_f32(inputs["fc1_w"]))                 # [512,4096]
    A = fw1.T.reshape(64, 64, 512)                       # [c][yx][m]
    Bm = A.reshape(64, 32, 2, 512)                       # [c][kt][yxp][m]
    fc1_st = np.ascontiguousarray(
        Bm.transpose(1, 2, 0, 3).reshape(32, 128, 512))  # [kt][kp][m]
    fw2 = np.sign(_f32(inputs["fc2_w"]))                 # [256,512]
    fc2_st = np.ascontiguousarray(fw2.T.reshape(4, 128, 256))

    wpk = np.zeros((128, 2176), np.uint8)
    for st, off, nkt, nm in ((fc1_st, 0, 32, 512), (fc2_st, 2048, 4, 256)):
        bits = (st > 0).astype(np.uint8)
        for b in range(8):
            wpk[:, off:off + nkt * nm // 8] |= (
                bits[:, :, b::8].transpose(1, 0, 2).reshape(128, -1)
                << np.uint8(b))

    # weights ship 8-way column-sharded and AllGather on device; x ships
    # one arg per chunk — the axon tunnel parallelizes per-arg transfers,
    # so 32 small args upload ~4x faster than 2 big ones
    w2v = np.ascontiguousarray(
        w2_st.transpose(1, 0, 2)).reshape(96, 192)      # [p][dy*64+m]
    in_maps = []
    for ci in range(NCORES):
        m = {
            "spk": np.ascontiguousarray(wpk[:, 272 * ci:272 * (ci + 1)]),
            "sbf": np.ascontiguousarray(wbf[:, 21 * ci:21 * (ci + 1)]),
            "sw2": np.ascontiguousarray(w2v[:, 24 * ci:24 * (ci + 1)]),
            "sct": np.ascontiguousarray(cst[:, 4 * ci:4 * (ci + 1)]),
        }
        for k in range(NCHUNK):
            m[f"xh{k}"] = np.ascontiguousarray(xh[ci, k])
            m[f"xl{k}"] = np.ascontiguousarray(xl[ci, k])
        in_maps.append(m)
    return in_maps


def _build_module(reps=1):
    import concourse.bass as bass
    import concourse.mybir as mybir
    import concourse.tile as tile
    from concourse import bacc
    from contextlib import ExitStack

    F32 = mybir.dt.float32
    BF16 = mybir.dt.bfloat16
    FP8 = mybir.dt.float8e4
    U8 = mybir.dt.uint8
    AF = mybir.ActivationFunctionType
    ALU = mybir.AluOpType

    nc = bacc.Bacc("TRN2", target_bir_lowering=False, debug=False,
                   num_devices=NCORES)

    U16 = mybir.dt.uint16
    RG = [list(range(NCORES))]

    # ---- DRAM tensors: weights arrive 8-way sharded, AllGather on-device;
    # x arrives as one tensor per chunk (parallel tunnel streams) ----
    d_xh = [nc.dram_tensor(f"xh{k}", [XCHW], U16, kind="ExternalInput")
            for k in range(NCHUNK)]
    d_xl = [nc.dram_tensor(f"xl{k}", [XCHM], U8, kind="ExternalInput")
            for k in range(NCHUNK)]
    shards = {}
    for nm, shp, dt in (("pk", [128, 272], U8), ("bf", [128, 21], BF16),
                        ("w2", [96, 24], FP8), ("ct", [128, 4], F32)):
        d_s = nc.dram_tensor("s" + nm, shp, dt, kind="ExternalInput")
        d_i = nc.dram_tensor("i" + nm, shp, dt, kind="Internal")
        d_g = nc.dram_tensor("g" + nm, [NCORES] + shp, dt, kind="Internal",
                             addr_space="Shared")
        shards[nm] = (d_s, d_i, d_g)
    d_out = nc.dram_tensor("out", [BC, 10], F32, kind="ExternalOutput")

    NBUF = 3                       # sp/ic2 ping-pong depth
    NXB = 2                        # ic1 staging ping-pong depth
    SPW = 4 * CH * 18              # sp payload per partition
    SP_FREE = SPW + 8
    IC2_FREE = 18 * CH * 18 + 8    # (R 18, s CH, W 18)

    with tile.TileContext(nc) as tc, ExitStack() as ctx:
        const = ctx.enter_context(tc.tile_pool(name="const", bufs=1))
        wk = ctx.enter_context(tc.tile_pool(name="wk", bufs=3))
        pp1 = ctx.enter_context(tc.tile_pool(name="pp1", bufs=3, space="PSUM"))
        pp2 = ctx.enter_context(tc.tile_pool(name="pp2", bufs=2, space="PSUM"))

        # ---- persistent tiles ----
        wbf_sb = const.tile([128, 168], BF16, tag="wbf")
        w2_sb = const.tile([96, 3, 64], FP8, tag="w2")
        fc1w_sb = const.tile([128, 32, 512], FP8, tag="fc1w")
        fc2w_sb = const.tile([128, 4, 256], FP8, tag="fc2w")
        cst_sb = const.tile([128, 32], F32, tag="cst")
        pk_sb = const.tile([128, 2176], U8, tag="pk")
        tmp_sb = const.tile([128, 2176], U8, tag="tmp")
        qh_t = [const.tile([128, 4096], U16, tag=f"qh{i}", name=f"qh{i}")
                for i in range(NXB)]
        qm_t = [const.tile([128, 2048], U8, tag=f"qm{i}", name=f"qm{i}")
                for i in range(NXB)]
        qmu_sb = const.tile([128, 4096], U8, tag="qmu")
        ich_t = [const.tile([128, 4096], BF16, tag=f"ich{i}", name=f"ich{i}")
                 for i in range(NXB)]
        icl_t = [const.tile([128, 4096], BF16, tag=f"icl{i}", name=f"icl{i}")
                 for i in range(NXB)]
        tf_sb = const.tile([128, 4096], F32, tag="tf")
        tm_sb = const.tile([128, 4096], F32, tag="tm")
        sp_t = [const.tile([128, SP_FREE], FP8, tag=f"sp{i}", name=f"sp{i}")
                for i in range(NBUF)]
        ic2_t = [const.tile([96, IC2_FREE], FP8, tag=f"ic2_{i}", name=f"ic2_{i}")
                 for i in range(NBUF)]
        # s2all: partition (g2, c64), free = yx*128 + sh,
        #   sh = (2*chk + tau)*4 + sl,  sample s = 8*(sh//4) + 4g + sh%4
        s2all = const.tile([128, 64 * 128], FP8, tag="s2all")
        # fc1in: partition kp = c + 64*(yx%2), free = kt*BC + (g*128 + sh)
        fc1in = const.tile([128, 32 * BC], FP8, tag="fc1in")
        s3_sb = const.tile([128, 4, BC], FP8, tag="s3")
        u4_sb = const.tile([128, 2, BC], F32, tag="u4")
        s4h_sb = const.tile([128, 2, BC], BF16, tag="s4h")
        s4l_sb = const.tile([128, 2, BC], BF16, tag="s4l")
        s4r_sb = const.tile([128, 2, BC], F32, tag="s4r")

        w1v = wbf_sb[:, 0:128]

        def ap_of(t, dims, off=0):
            return bass.AP(tensor=t.tensor, offset=t.offset + off,
                           ap=[list(t.ap[0])] + [list(d) for d in dims])

        # ---- setup: AllGather the weight shards, then load to SBUF ----
        for nm, (d_s, d_i, d_g) in shards.items():
            nc.gpsimd.dma_start(d_i.ap(), d_s.ap())
            nc.gpsimd.collective_compute(
                "AllGather", ALU.bypass, RG, ins=[d_i.ap()], outs=[d_g.ap()])
        for c in range(NCORES):
            g = shards["pk"][2].ap()[c]
            nc.gpsimd.dma_start(pk_sb[:, 272 * c:272 * (c + 1)], g)
            g = shards["bf"][2].ap()[c]
            nc.gpsimd.dma_start(wbf_sb[:, 21 * c:21 * (c + 1)], g)
            g = shards["w2"][2].ap()[c]
            nc.gpsimd.dma_start(bass.AP(
                tensor=w2_sb.tensor, offset=w2_sb.offset + 24 * c,
                ap=[list(w2_sb.ap[0]), [1, 24]]), g)
            g = shards["ct"][2].ap()[c]
            nc.gpsimd.dma_start(cst_sb[:, 4 * c:4 * (c + 1)], g)

        # fc1w/fc2w unpack: bit b of packed byte (kt,mb) -> m = 8*mb + b
        for b in range(8):
            nc.vector.tensor_scalar(tmp_sb[:], pk_sb[:], b, 1,
                                    ALU.logical_shift_right, ALU.bitwise_and)
            dst = bass.AP(tensor=fc1w_sb.tensor, offset=fc1w_sb.offset + b,
                          ap=[list(fc1w_sb.ap[0]), [512, 32], [8, 64]])
            nc.vector.tensor_scalar(dst, bass.AP(
                tensor=tmp_sb.tensor, offset=tmp_sb.offset,
                ap=[list(tmp_sb.ap[0]), [1, 2048]]), 2.0, -1.0,
                ALU.mult, ALU.add)
            dst2 = bass.AP(tensor=fc2w_sb.tensor, offset=fc2w_sb.offset + b,
                           ap=[list(fc2w_sb.ap[0]), [256, 4], [8, 32]])
            nc.vector.tensor_scalar(dst2, bass.AP(
                tensor=tmp_sb.tensor, offset=tmp_sb.offset + 2048,
                ap=[list(tmp_sb.ap[0]), [1, 128]]), 2.0, -1.0,
                ALU.mult, ALU.add)

        # init the gather tiles to the encoding of 0.0 once: pad partitions
        # and clipped row slabs stay at q0 forever and decode to exact 0.
        for t in qh_t:
            nc.vector.memset(t[:], Q0 >> 4)
        for t in qm_t:
            nc.vector.memset(t[:], 0)
        # sp pads: cols w==0 and w==17 of each 18-block, plus the 8 slack cols
        for t in sp_t:
            nc.vector.memset(
                ap_of(t, [[18, SPW // 18], [17, 2], [1, 1]]), 0.0)
            nc.vector.memset(ap_of(t, [[1, 8]], SPW), 0.0)
        # ic2 pads: halo rows R=0 and R=17 (+ slack)
        for t in ic2_t:
            nc.vector.memset(
                ap_of(t, [[17 * CH * 18, 2], [1, CH * 18]]), 0.0)
            nc.vector.memset(ap_of(t, [[1, 8]], 18 * CH * 18), 0.0)

        dma_engs = [nc.sync, nc.scalar, nc.gpsimd]

        for _rep in range(reps):
            # ================= chunk loop (software-pipelined) =================
            # iteration k emits conv1 of chunk k, then conv2 of chunk k-1, so
            # the in-order PE queue never stalls on chunk k-1's ic2 DMAs.
            def conv1_part(chk):
                sp = sp_t[chk % NBUF]
                ic2 = ic2_t[chk % NBUF]
                qh = qh_t[chk % NXB]
                qm = qm_t[chk % NXB]
                ich = ich_t[chk % NXB]
                icl = icl_t[chk % NXB]

                # ---- on-device im2col: 36 gather DMAs per half ----
                # dst w-range / src col-range clip at the image edge; the
                # unwritten halo positions keep their q0 memset (decode 0).
                qi = 0
                for tgt, dsrc, G in ((qh, d_xh[chk], 16), (qm, d_xl[chk], 8)):
                    for j in range(4):
                        for dy in range(3):
                            r0, nr = 0, 8
                            if j == 0 and dy == 0:
                                r0, nr = 1, 7
                            if j == 3 and dy == 2:
                                r0, nr = 0, 7
                            soff = (8 * j + dy + r0 - 1) * (32 * G)
                            for dx in range(3):
                                p0 = 32 * j + 9 * dy + 3 * dx
                                run = 32 * G if dx == 1 else 31 * G
                                pv = tgt[p0:p0 + 3]
                                dst = bass.AP(
                                    tensor=pv.tensor,
                                    offset=pv.offset + r0 * 32 * G
                                    + (G if dx == 0 else 0),
                                    ap=[list(pv.ap[0]), [32 * G, nr],
                                        [1, run]])
                                src = bass.AP(
                                    tensor=dsrc,
                                    offset=soff + (G if dx == 2 else 0),
                                    ap=[[1024 * G, 3], [32 * G, nr],
                                        [1, run]])
                                dma_engs[qi % 3].dma_start(dst, src)
                                qi += 1

                # ---- nibble unpack + int20 decode + bf16 hi/lo split ----
                nc.vector.tensor_scalar(
                    ap_of(qmu_sb, [[16, 256], [2, 8]]), qm[:],
                    15, None, ALU.bitwise_and)
                nc.vector.tensor_scalar(
                    ap_of(qmu_sb, [[16, 256], [2, 8]], 1), qm[:],
                    4, None, ALU.logical_shift_right)
                nc.vector.tensor_scalar(tf_sb[:], qh[:], C_HI, None, ALU.mult)
                nc.vector.tensor_scalar(tm_sb[:], qmu_sb[:], C_LO, -6.0,
                                        ALU.mult, ALU.add)
                nc.vector.tensor_add(tf_sb[:], tf_sb[:], tm_sb[:])
                nc.vector.tensor_copy(ich[:], tf_sb[:])
                nc.vector.tensor_sub(tm_sb[:], tf_sb[:], ich[:])
                nc.vector.tensor_copy(icl[:], tm_sb[:])

                # ---- conv1: 4 psum tiles; tile tau = pooled row pair ----
                # psum free = dr*512 + w*16 + s
                for tau in range(4):
                    p1 = pp1.tile([128, 1024], F32, tag="c1")
                    for dr in range(2):
                        sl = bass.ds((2 * tau + dr) * 512, 512)
                        nc.tensor.matmul(p1[:, dr * 512:(dr + 1) * 512],
                                         w1v, ich[:, sl],
                                         start=True, stop=False)
                        nc.tensor.matmul(p1[:, dr * 512:(dr + 1) * 512],
                                         w1v, icl[:, sl],
                                         start=False, stop=True)
                    # DVE: 2x2 maxpool in ONE XY-window reduce from psum
                    # out free = 16*s + wp
                    pl1 = wk.tile([128, 256], F32, tag="pl1")
                    nc.vector.tensor_reduce(
                        ap_of(pl1, [[16, 16], [1, 16]]),
                        ap_of(p1, [[1, 16], [32, 16], [512, 2], [16, 2]]),
                        mybir.AxisListType.XY, ALU.max)
                    # ACT: sign(bn1) -> +-1 fp8 straight into sp
                    # sp free = rr*288 + s*18 + (wp+1), rr = tau
                    nc.scalar.activation(
                        ap_of(sp, [[18, 16], [1, 16]], tau * 288 + 1),
                        pl1[:], AF.Sign,
                        bias=cst_sb[:, 1:2], scale=cst_sb[:, 0:1])

                # ---- ic2 build: 3 contiguous-run DMAs (SP, ACT, Pool) ----
                RUN = 4 * CH * 18
                for dx in range(3):
                    src = bass.AP(tensor=sp.tensor, offset=sp.offset + dx,
                                  ap=[list(sp.ap[0]), [1, RUN]])
                    dst_t = ic2[32 * dx:32 * (dx + 1)]
                    dst = bass.AP(tensor=dst_t.tensor,
                                  offset=dst_t.offset + CH * 18,
                                  ap=[list(dst_t.ap[0]), [RUN, 4], [1, RUN]])
                    eng = (nc.sync, nc.scalar, nc.gpsimd)[dx]
                    eng.dma_start(dst, src)

            def conv2_part(chk):
                ic2 = ic2_t[chk % NBUF]
                # ---- conv2: 4 one-bank col-packed psum tiles (tau, h) ----
                for tau in range(2):
                    for h in range(2):
                        p2 = pp2.tile([128, 512], F32, tag="c2")
                        for g in range(2):
                            tp = (0, 64 * g) if g else None
                            for dy in range(3):
                                s0 = 8 * tau + 4 * g + 2 * h
                                mv = bass.AP(
                                    tensor=ic2.tensor,
                                    offset=ic2.offset + s0 * 18
                                    + dy * (CH * 18),
                                    ap=[list(ic2.ap[0]), [18, 2],
                                        [CH * 18, 16], [1, 16]])
                                nc.tensor.matmul(
                                    p2[64 * g:64 * (g + 1), :],
                                    w2_sb[:, dy, :], mv,
                                    start=(dy == 0), stop=(dy == 2),
                                    tile_position=tp)
                        # DVE: 2x2 maxpool, one XY-window reduce
                        xm2b = wk.tile([128, 128], F32, tag="xm2b")
                        nc.vector.tensor_reduce(
                            ap_of(xm2b, [[8, 16], [1, 8]]),
                            ap_of(p2, [[32, 16], [2, 8], [16, 2], [1, 2]]),
                            mybir.AxisListType.XY, ALU.max)
                        # ACT sign(bn2) -> +-1 fp8 into s2all
                        sh0 = (2 * chk + tau) * 4 + 2 * h
                        nc.scalar.activation(
                            ap_of(s2all, [[1, 2], [1024, 8], [128, 8]], sh0),
                            xm2b[:], AF.Sign,
                            bias=cst_sb[:, 3:4], scale=cst_sb[:, 2:3])

                # ---- repack waves: one DMA per (g, yxp, wave) ----
                # waves at chk 7 (sh 0:64), 11 (64:96), 15 (96:128) so the
                # final fc1 dependency is only a quarter-size transfer
                WAVES = {7: (0, 64), 11: (64, 32), 15: (96, 32)}
                if chk in WAVES:
                    lo, ln = WAVES[chk]
                    for g in range(2):
                        for yxp in range(2):
                            src_t = s2all[64 * g:64 * (g + 1)]
                            src = bass.AP(
                                tensor=src_t.tensor,
                                offset=src_t.offset + 128 * yxp + lo,
                                ap=[list(src_t.ap[0]), [256, 32], [1, ln]])
                            dst_t = fc1in[64 * yxp:64 * (yxp + 1)]
                            dst = bass.AP(
                                tensor=dst_t.tensor,
                                offset=dst_t.offset + g * 128 + lo,
                                ap=[list(dst_t.ap[0]), [256, 32], [1, ln]])
                            eng = (nc.sync, nc.scalar, nc.gpsimd,
                                   nc.gpsimd)[2 * g + yxp]
                            eng.dma_start(dst, src)

            conv1_part(0)
            for chk in range(1, NCHUNK):
                conv1_part(chk)
                conv2_part(chk - 1)
            conv2_part(NCHUNK - 1)

            # ================= fc phase =================
            # fc1: 4 m-tiles, fp8 DoubleRow over kt pairs, column-sliced:
            # cols filled by waves 1+2 (sh 0:96 of both g-blocks) run while
            # wave 3 is still transferring; the wave-3 cols follow.
            pfs = []
            for m in range(4):
                pf = (pp1 if m < 3 else pp2).tile(
                    [128, 1024] if m < 3 else [128, 512], F32,
                    tag="c1" if m < 3 else "c2", name=f"pf_{m}")
                pfs.append(pf)
            SLICES = [[(0, 96), (128, 96)], [(96, 32), (224, 32)]]
            for phase in range(2):
                for m in range(4):
                    pf = pfs[m]
                    for c0, wdt in SLICES[phase]:
                        for kt in range(16):
                            lw = bass.AP(
                                tensor=fc1w_sb.tensor,
                                offset=fc1w_sb.offset + 2 * kt * 512
                                + 128 * m,
                                ap=[list(fc1w_sb.ap[0]), [512, 2], [1, 128]])
                            mv = bass.AP(
                                tensor=fc1in.tensor,
                                offset=fc1in.offset + 2 * kt * BC + c0,
                                ap=[list(fc1in.ap[0]), [BC, 2], [1, wdt]])
                            nc.tensor.matmul(
                                pf[:, c0:c0 + wdt], lw, mv,
                                start=(kt == 0), stop=(kt == 15),
                                perf_mode=mybir.MatmulPerfMode.DoubleRow)
            for m in range(4):
                nc.scalar.activation(s3_sb[:, m, :], pfs[m][:, :BC], AF.Sign,
                                     bias=cst_sb[:, 8 + m:9 + m],
                                     scale=cst_sb[:, 4 + m:5 + m])

            # fc2: 2 m-tiles, fp8 DoubleRow over kt pairs; clip + hi/lo
            # split per m2-half so the DVE chain overlaps fc2's second half
            for m2 in range(2):
                pg = pp1.tile([128, 1024], F32, tag="c1")
                for kt in range(2):
                    lw = bass.AP(
                        tensor=fc2w_sb.tensor,
                        offset=fc2w_sb.offset + 2 * kt * 256 + 128 * m2,
                        ap=[list(fc2w_sb.ap[0]), [256, 2], [1, 128]])
                    mv = bass.AP(
                        tensor=s3_sb.tensor,
                        offset=s3_sb.offset + 2 * kt * BC,
                        ap=[list(s3_sb.ap[0]), [BC, 2], [1, BC]])
                    nc.tensor.matmul(pg[:, :BC], lw, mv,
                                     start=(kt == 0), stop=(kt == 1),
                                     perf_mode=mybir.MatmulPerfMode.DoubleRow)
                nc.scalar.activation(u4_sb[:, m2, :], pg[:, :BC], AF.Identity,
                                     bias=cst_sb[:, 14 + m2:15 + m2],
                                     scale=cst_sb[:, 12 + m2:13 + m2])
                nc.vector.tensor_scalar(u4_sb[:, m2, :], u4_sb[:, m2, :],
                                        1.0, -1.0, ALU.min, ALU.max)
                nc.vector.tensor_copy(s4h_sb[:, m2, :], u4_sb[:, m2, :])
                nc.vector.tensor_sub(s4r_sb[:, m2, :], u4_sb[:, m2, :],
                                     s4h_sb[:, m2, :])
                nc.vector.tensor_copy(s4l_sb[:, m2, :], s4r_sb[:, m2, :])

            # fc3 + log_softmax; batch tile bt == g block of fc1 columns.
            h3s, mxs, negs, ses, lss = [], [], [], [], []
            for bt in range(2):
                ph = pp2.tile([128, 512], F32, tag="c2")
                mms = []
                for kt in range(2):
                    lh = s4h_sb[:, kt, 128 * bt:128 * (bt + 1)]
                    ll = s4l_sb[:, kt, 128 * bt:128 * (bt + 1)]
                    w3hv = wbf_sb[:, 128 + 10 * kt:138 + 10 * kt]
                    w3lv = wbf_sb[:, 148 + 10 * kt:158 + 10 * kt]
                    mms += [(lh, w3hv), (ll, w3hv), (lh, w3lv)]
                for i, (lhs, rhs) in enumerate(mms):
                    nc.tensor.matmul(ph[:, :10], lhs, rhs,
                                     start=(i == 0), stop=(i == len(mms) - 1))
                h3 = wk.tile([128, 10], F32, tag="h3", name=f"h3_{bt}")
                nc.vector.tensor_add(h3[:], ph[:, :10], cst_sb[:, 16:26])
                mx = wk.tile([128, 1], F32, tag="mx", name=f"mx_{bt}")
                nc.vector.tensor_reduce(mx[:], h3[:], mybir.AxisListType.X,
                                        ALU.max)
                negmx = wk.tile([128, 1], F32, tag="negmx", name=f"negmx_{bt}")
                nc.vector.tensor_scalar_mul(negmx[:], mx[:], -1.0)
                h3s.append(h3)
                mxs.append(mx)
                negs.append(negmx)
            for bt in range(2):     # both Exps together (one act table set)
                et = wk.tile([128, 10], F32, tag="et", name=f"et_{bt}")
                se = wk.tile([128, 1], F32, tag="se", name=f"se_{bt}")
                nc.scalar.activation(et[:], h3s[bt][:], AF.Exp,
                                     bias=negs[bt][:], scale=1.0,
                                     accum_out=se[:])
                ses.append(se)
            for bt in range(2):     # then both Lns (single table reload)
                ls = wk.tile([128, 1], F32, tag="ls", name=f"ls_{bt}")
                nc.scalar.activation(ls[:], ses[bt][:], AF.Ln)
                lss.append(ls)
            for bt in range(2):
                tt = wk.tile([128, 1], F32, tag="tt", name=f"tt_{bt}")
                nc.vector.tensor_add(tt[:], mxs[bt][:], lss[bt][:])
                o = wk.tile([128, 10], F32, tag="o", name=f"o_{bt}")
                nc.vector.tensor_scalar_sub(o[:], h3s[bt][:], tt[:])
                # un-permute: psum partition p -> sample row
                # s = 8*(p//4) + 4*bt + p%4
                dst = bass.AP(tensor=d_out, offset=bt * 40,
                              ap=[[80, 32], [10, 4], [1, 10]])
                (nc.sync if bt == 0 else nc.scalar).dma_start(dst, o[:])

    nc.compile()
    return nc


def _get_module():
    if "nc" not in _nc_cache:
        nc = _build_module()
        # the module is frozen after compile; memoize its (identical)
        # serialization so per-call re-jits don't re-serialize ~30ms worth
        js = nc.to_json_bytes()
        nc.to_json_bytes = lambda: js
        _nc_cache["nc"] = nc
    return _nc_cache["nc"]


def kernel(**inputs):
    from concourse.bass_utils import run_bass_kernel_spmd

    in_maps = _host_prep(inputs)
    nc = _get_module()
    res = run_bass_kernel_spmd(nc, in_maps, core_ids=list(range(NCORES)))
    out = np.concatenate([r["out"] for r in res.results], axis=0)
    return out.astype(np.float32)


# revision 41
# speedup vs baseline: 1.2834x; 1.2834x over previous
"""BinarizedLeNet5/CIFAR10 Trainium2 kernel (8-core data parallel), v4.

The graded metric is wall-clock of a warm run_bass_kernel_spmd call, and
the axon tunnel moves ~50 MB/s — so v2's 287 MB of host-staged im2col
inputs WAS the runtime.  v4 ships ~16.5 MB instead:

- x goes up as int20 fixed-point over [-6,6): u16 hi + nibble-packed u4
  mantissa extension, unpadded, samples innermost.  The im2col expansion
  happens on-device with 36 strided gather DMAs per chunk-half (3-dim
  APs, contiguous (col,s) runs); halo positions keep a one-time q0
  memset that decodes to exact 0.  DVE unpacks the nibbles, decodes to
  f32 and splits into the bf16 hi/lo pair the conv1 matmuls consume
  (the split residual ~x*2^-18 dominates the encoding error; rel err
  1.12e-2 vs the 2e-2 gate).  conv1 psum free layout is (r, w, s).
- fc1w+fc2w ship bit-packed in one [128,2176] u8 tensor, unpacked
  on-device with DVE shift/and ops.
- all weight tensors arrive 8-way column-sharded and are AllGather'd
  on-device (replicated upload would cost 8x the bytes).
- small f32 constants consolidate into one [128,32] tensor, w1/w3h/w3l
  into one [128,168] bf16 tensor: 6 input args total.

Everything downstream of conv1's ACT-sign (sp/ic2 build, conv2, fc
phase, log_softmax tail) is byte-identical to v2.  A persistent XLA
compilation cache is configured at import so warm calls skip the
~0.5s client-side BIR re-verification that run_bass_kernel_spmd's
per-call re-jit otherwise pays.
"""
import sys
import numpy as np

sys.path.insert(0, "/opt/pypackages")
sys.path.insert(0, "/opt/trn_rl_repo")

import ml_dtypes

# Persistent XLA compilation cache: run_bass_kernel_spmd re-jits on every
# call (fresh closure), and without this each warm call pays ~0.5s of
# client-side BIR re-verification + DVE table gen before the NEFF cache
# hits.  The persistent cache keys on HLO fingerprint, so call 2+ skips
# backend compile entirely.
import jax

jax.config.update("jax_compilation_cache_dir", "/tmp/jax_comp_cache")
jax.config.update("jax_persistent_cache_min_entry_size_bytes", -1)
jax.config.update("jax_persistent_cache_min_compile_time_secs", 0.0)

BF = ml_dtypes.bfloat16
F8 = ml_dtypes.float8_e4m3
NCORES = 8
B = 2048
BC = B // NCORES          # 256 samples per core
CH = 16                   # samples per chunk
NCHUNK = BC // CH         # 16 chunks
EPS = np.float32(1e-5)
XCHW = 3 * 32 * 32 * CH   # chunk elements, unpadded (49152)
XCHM = XCHW // 2          # nibble-packed chunk bytes (24576)
XSC = float(2 ** 20) / 12.0   # int20 fixed-point scale over [-6, 6)
Q0 = 1 << 19                  # encoding of x == 0.0
C_HI = 3.0 * 2.0 ** -14       # decode: x = hi16*C_HI + m4*C_LO - 6
C_LO = 3.0 * 2.0 ** -18

_nc_cache = {}


def _f32(x):
    return np.asarray(x, np.float32)


def _host_prep(inputs):
    """Build all per-core device input arrays (layout prep only)."""
    x = _f32(inputs["x"])                      # [2048,3,32,32]

    # ---- x as int20 fixed-point over [-6,6): u16 hi + u4 nibble ext,
    # unpadded, samples innermost (halo positions stay at the q0 memset) ----
    q = np.clip(np.round((x.astype(np.float64) + 6.0) * XSC),
                0, 2 ** 20 - 1).astype(np.uint32)

    def stage(xq):
        xt = xq.reshape(NCORES, NCHUNK, CH, 3, 32, 32)
        xt = np.ascontiguousarray(xt.transpose(0, 1, 3, 4, 5, 2))
        return xt.reshape(NCORES, NCHUNK, -1)

    xh = stage((q >> 4).astype(np.uint16))
    m4 = stage((q & 15).astype(np.uint8))       # [.., pos, s]
    m4 = m4.reshape(NCORES, NCHUNK, XCHM, 2)    # byte = (s even, s odd)
    xl = (m4[..., 0] | (m4[..., 1] << 4)).astype(np.uint8)
    # one u16 arg per core: [hi16 | nibble bytes viewed as u16 pairs]
    xq = np.concatenate(
        [xh, np.ascontiguousarray(xl).view(np.uint16)], axis=2)

    # ---- conv1 stationary: block-diag, k = 3*(3dy+dx)+c, out p = 4*co+j ----
    w1s = np.sign(_f32(inputs["conv1_w"]))               # [32,3,3,3]
    w1k = np.ascontiguousarray(w1s.transpose(2, 3, 1, 0)).reshape(27, 32)
    # bf16 blob: [:,0:128]=w1, [:,128:148]=w3h (kt-major), [:,148:168]=w3l
    wbf = np.zeros((128, 168), BF)
    for j in range(4):
        for co in range(32):
            wbf[32 * j:32 * j + 27, 4 * co + j] = w1k[:, co].astype(BF)
    w3 = _f32(inputs["fc3_w"]).T                         # [256,10]
    w3h = w3.astype(BF)
    w3l = (w3 - w3h.astype(np.float32)).astype(BF)
    for kt in range(2):
        wbf[:, 128 + 10 * kt:138 + 10 * kt] = w3h[128 * kt:128 * (kt + 1)]
        wbf[:, 148 + 10 * kt:158 + 10 * kt] = w3l[128 * kt:128 * (kt + 1)]

    # ---- conv2 stationaries [3][96,64]: p = 32*dx + c (bit-packed) ----
    w2s = np.sign(_f32(inputs["conv2_w"]))               # [64,32,3,3]
    w2_st = np.zeros((3, 96, 64), np.float32)
    for dy in range(3):
        for dx in range(3):
            w2_st[dy, 32 * dx:32 * dx + 32] = w2s[:, :, dy, dx].T

    # ---- consolidated f32 constants [128, 32] (padded for 8-way shard) ----
    cst = np.zeros((128, 32), np.float32)
    inv1 = _f32(inputs["bn1_g"]) / np.sqrt(_f32(inputs["bn1_v"]) + EPS)
    sh1c = (_f32(inputs["conv1_b"]) - _f32(inputs["bn1_m"])) * inv1 \
        + _f32(inputs["bn1_b"])
    cst[:, 0] = np.repeat(inv1, 4)
    cst[:, 1] = np.repeat(sh1c, 4)
    inv2 = _f32(inputs["bn2_g"]) / np.sqrt(_f32(inputs["bn2_v"]) + EPS)
    sh2c = _f32(inputs["bn2_b"]) - _f32(inputs["bn2_m"]) * inv2
    cst[:, 2] = np.tile(inv2, 2)
    cst[:, 3] = np.tile(_f32(inputs["conv2_b"]) * inv2 + sh2c, 2)
    inv3 = _f32(inputs["bn3_g"]) / np.sqrt(_f32(inputs["bn3_v"]) + EPS)
    sh3c = (_f32(inputs["fc1_b"]) - _f32(inputs["bn3_m"])) * inv3 \
        + _f32(inputs["bn3_b"])
    cst[:, 4:8] = inv3.reshape(4, 128).T
    cst[:, 8:12] = sh3c.reshape(4, 128).T
    inv4 = _f32(inputs["bn4_g"]) / np.sqrt(_f32(inputs["bn4_v"]) + EPS)
    sh4c = (_f32(inputs["fc2_b"]) - _f32(inputs["bn4_m"])) * inv4 \
        + _f32(inputs["bn4_b"])
    cst[:, 12:14] = inv4.reshape(2, 128).T
    cst[:, 14:16] = sh4c.reshape(2, 128).T
    cst[:, 16:26] = _f32(inputs["fc3_b"])[None, :]

    # ---- fc1+fc2+w2 bit-packed into one [128, 2200] u8 tensor ----
    # fc1: kt = yx//2, kp = c + 64*(yx%2), bits along m (cols 0:2048);
    # fc2 at 2048:2176; w2 (rows 0:96 only) at 2176:2200
    fw1 = np.sign(_f32(inputs["fc1_w"]))                 # [512,4096]
    A = fw1.T.reshape(64, 64, 512)                       # [c][yx][m]
    Bm = A.reshape(64, 32, 2, 512)                       # [c][kt][yxp][m]
    fc1_st = np.ascontiguousarray(
        Bm.transpose(1, 2, 0, 3).reshape(32, 128, 512))  # [kt][kp][m]
    fw2 = np.sign(_f32(inputs["fc2_w"]))                 # [256,512]
    fc2_st = np.ascontiguousarray(fw2.T.reshape(4, 128, 256))
    w2v = np.zeros((128, 192), np.float32)               # [p][dy*64+m]
    w2v[:96] = np.ascontiguousarray(
        w2_st.transpose(1, 0, 2)).reshape(96, 192)

    wpk = np.zeros((128, 2200), np.uint8)
    for st, off in ((fc1_st.transpose(1, 0, 2).reshape(128, -1), 0),
                    (fc2_st.transpose(1, 0, 2).reshape(128, -1), 2048),
                    (w2v, 2176)):
        bits = (st > 0).astype(np.uint8)
        for b in range(8):
            wpk[:, off:off + st.shape[1] // 8] |= (bits[:, b::8]
                                                   << np.uint8(b))

    # cst ships as a bf16 hi/lo pair alongside wbf (one bf16 arg)
    ch = cst.astype(BF)
    cl = (cst - ch.astype(np.float32)).astype(BF)
    sbf = np.concatenate([wbf, ch, cl], axis=1)          # [128, 232]

    # weights ship 8-way column-sharded and AllGather on device; x ships
    # as ONE arg per core (fewer args = less per-transfer overhead)
    in_maps = []
    for ci in range(NCORES):
        m = {
            "xq": np.ascontiguousarray(xq[ci]),
            "s8": np.ascontiguousarray(wpk[:, 275 * ci:275 * (ci + 1)]),
            "sbf": np.ascontiguousarray(sbf[:, 29 * ci:29 * (ci + 1)]),
        }
        in_maps.append(m)
    return in_maps


def _build_module(reps=1):
    import concourse.bass as bass
    import concourse.mybir as mybir
    import concourse.tile as tile
    from concourse import bacc
    from contextlib import ExitStack

    F32 = mybir.dt.float32
    BF16 = mybir.dt.bfloat16
    FP8 = mybir.dt.float8e4
    U8 = mybir.dt.uint8
    AF = mybir.ActivationFunctionType
    ALU = mybir.AluOpType

    nc = bacc.Bacc("TRN2", target_bir_lowering=False, debug=False,
                   num_devices=NCORES)

    U16 = mybir.dt.uint16
    RG = [list(range(NCORES))]

    # ---- DRAM tensors: weights arrive 8-way sharded, AllGather on-device;
    # x is one u16 tensor per core: [hi16 | nibble-pairs] per chunk row ----
    XROW = XCHW + XCHM // 2
    d_xq = nc.dram_tensor("xq", [NCHUNK, XROW], U16, kind="ExternalInput")
    shards = {}
    for nm, shp, dt in (("s8", [128, 275], U8), ("sbf", [128, 29], BF16)):
        d_s = nc.dram_tensor(nm, shp, dt, kind="ExternalInput")
        d_i = nc.dram_tensor("i" + nm, shp, dt, kind="Internal")
        d_g = nc.dram_tensor("g" + nm, [NCORES] + shp, dt, kind="Internal",
                             addr_space="Shared")
        shards[nm] = (d_s, d_i, d_g)
    d_out = nc.dram_tensor("out", [BC, 10], F32, kind="ExternalOutput")

    NBUF = 3                       # sp/ic2 ping-pong depth
    NXB = 2                        # ic1 staging ping-pong depth
    SPW = 4 * CH * 18              # sp payload per partition
    SP_FREE = SPW + 8
    IC2_FREE = 18 * CH * 18 + 8    # (R 18, s CH, W 18)

    with tile.TileContext(nc) as tc, ExitStack() as ctx:
        const = ctx.enter_context(tc.tile_pool(name="const", bufs=1))
        wk = ctx.enter_context(tc.tile_pool(name="wk", bufs=3))
        pp1 = ctx.enter_context(tc.tile_pool(name="pp1", bufs=3, space="PSUM"))
        pp2 = ctx.enter_context(tc.tile_pool(name="pp2", bufs=2, space="PSUM"))

        # ---- persistent tiles ----
        sbf_sb = const.tile([128, 232], BF16, tag="sbf")
        w2_sb = const.tile([96, 3, 64], FP8, tag="w2")
        fc1w_sb = const.tile([128, 32, 512], FP8, tag="fc1w")
        fc2w_sb = const.tile([128, 4, 256], FP8, tag="fc2w")
        cst_sb = const.tile([128, 32], F32, tag="cst")
        pk_sb = const.tile([128, 2200], U8, tag="pk")
        tmp_sb = const.tile([128, 2200], U8, tag="tmp")
        qh_t = [const.tile([128, 4096], U16, tag=f"qh{i}", name=f"qh{i}")
                for i in range(NXB)]
        qm_t = [const.tile([128, 1024], U16, tag=f"qm{i}", name=f"qm{i}")
                for i in range(NXB)]
        qmu_sb = const.tile([128, 4096], U16, tag="qmu")
        ich_t = [const.tile([128, 4096], BF16, tag=f"ich{i}", name=f"ich{i}")
                 for i in range(NXB)]
        icl_t = [const.tile([128, 4096], BF16, tag=f"icl{i}", name=f"icl{i}")
                 for i in range(NXB)]
        tf_sb = const.tile([128, 4096], F32, tag="tf")
        tm_sb = const.tile([128, 4096], F32, tag="tm")
        sp_t = [const.tile([128, SP_FREE], FP8, tag=f"sp{i}", name=f"sp{i}")
                for i in range(NBUF)]
        ic2_t = [const.tile([96, IC2_FREE], FP8, tag=f"ic2_{i}", name=f"ic2_{i}")
                 for i in range(NBUF)]
        # s2all: partition (g2, c64), free = yx*128 + sh,
        #   sh = (2*chk + tau)*4 + sl,  sample s = 8*(sh//4) + 4g + sh%4
        s2all = const.tile([128, 64 * 128], FP8, tag="s2all")
        # fc1in: partition kp = c + 64*(yx%2), free = kt*BC + (g*128 + sh)
        fc1in = const.tile([128, 32 * BC], FP8, tag="fc1in")
        s3_sb = const.tile([128, 4, BC], FP8, tag="s3")
        u4_sb = const.tile([128, 2, BC], F32, tag="u4")
        s4h_sb = const.tile([128, 2, BC], BF16, tag="s4h")
        s4l_sb = const.tile([128, 2, BC], BF16, tag="s4l")
        s4r_sb = const.tile([128, 2, BC], F32, tag="s4r")

        w1v = sbf_sb[:, 0:128]

        def ap_of(t, dims, off=0):
            return bass.AP(tensor=t.tensor, offset=t.offset + off,
                           ap=[list(t.ap[0])] + [list(d) for d in dims])

        # ---- setup: AllGather the weight shards, then load to SBUF ----
        for nm, (d_s, d_i, d_g) in shards.items():
            nc.gpsimd.dma_start(d_i.ap(), d_s.ap())
            nc.gpsimd.collective_compute(
                "AllGather", ALU.bypass, RG, ins=[d_i.ap()], outs=[d_g.ap()])
        for c in range(NCORES):
            g = shards["s8"][2].ap()[c]
            nc.gpsimd.dma_start(pk_sb[:, 275 * c:275 * (c + 1)], g)
            g = shards["sbf"][2].ap()[c]
            nc.gpsimd.dma_start(sbf_sb[:, 29 * c:29 * (c + 1)], g)
        # cst = bf16 hi + bf16 lo
        nc.vector.tensor_add(cst_sb[:], sbf_sb[:, 168:200],
                             sbf_sb[:, 200:232])

        # fc1w/fc2w/w2 unpack: bit b of packed byte mb -> m = 8*mb + b
        for b in range(8):
            nc.vector.tensor_scalar(tmp_sb[:], pk_sb[:], b, 1,
                                    ALU.logical_shift_right, ALU.bitwise_and)
            dst = bass.AP(tensor=fc1w_sb.tensor, offset=fc1w_sb.offset + b,
                          ap=[list(fc1w_sb.ap[0]), [512, 32], [8, 64]])
            nc.vector.tensor_scalar(dst, bass.AP(
                tensor=tmp_sb.tensor, offset=tmp_sb.offset,
                ap=[list(tmp_sb.ap[0]), [1, 2048]]), 2.0, -1.0,
                ALU.mult, ALU.add)
            dst2 = bass.AP(tensor=fc2w_sb.tensor, offset=fc2w_sb.offset + b,
                           ap=[list(fc2w_sb.ap[0]), [256, 4], [8, 32]])
            nc.vector.tensor_scalar(dst2, bass.AP(
                tensor=tmp_sb.tensor, offset=tmp_sb.offset + 2048,
                ap=[list(tmp_sb.ap[0]), [1, 128]]), 2.0, -1.0,
                ALU.mult, ALU.add)
            dst3 = bass.AP(tensor=w2_sb.tensor, offset=w2_sb.offset + b,
                           ap=[list(w2_sb.ap[0]), [8, 24]])
            tv = tmp_sb[0:96]
            nc.vector.tensor_scalar(dst3, bass.AP(
                tensor=tv.tensor, offset=tv.offset + 2176,
                ap=[list(tv.ap[0]), [1, 24]]), 2.0, -1.0,
                ALU.mult, ALU.add)

        # init the gather tiles to the encoding of 0.0 once: pad partitions
        # and clipped row slabs stay at q0 forever and decode to exact 0.
        for t in qh_t:
            nc.vector.memset(t[:], Q0 >> 4)
        for t in qm_t:
            nc.vector.memset(t[:], 0)
        # sp pads: cols w==0 and w==17 of each 18-block, plus the 8 slack cols
        for t in sp_t:
            nc.vector.memset(
                ap_of(t, [[18, SPW // 18], [17, 2], [1, 1]]), 0.0)
            nc.vector.memset(ap_of(t, [[1, 8]], SPW), 0.0)
        # ic2 pads: halo rows R=0 and R=17 (+ slack)
        for t in ic2_t:
            nc.vector.memset(
                ap_of(t, [[17 * CH * 18, 2], [1, CH * 18]]), 0.0)
            nc.vector.memset(ap_of(t, [[1, 8]], 18 * CH * 18), 0.0)

        dma_engs = [nc.sync, nc.scalar, nc.gpsimd]

        for _rep in range(reps):
            # ================= chunk loop (software-pipelined) =================
            # iteration k emits conv1 of chunk k, then conv2 of chunk k-1, so
            # the in-order PE queue never stalls on chunk k-1's ic2 DMAs.
            def conv1_part(chk):
                sp = sp_t[chk % NBUF]
                ic2 = ic2_t[chk % NBUF]
                qh = qh_t[chk % NXB]
                qm = qm_t[chk % NXB]
                ich = ich_t[chk % NXB]
                icl = icl_t[chk % NXB]

                # ---- on-device im2col: 36 gather DMAs per half ----
                # dst w-range / src col-range clip at the image edge; the
                # unwritten halo positions keep their q0 memset (decode 0).
                qi = 0
                for tgt, base, G in ((qh, 0, 16), (qm, XCHW, 4)):
                    for j in range(4):
                        for dy in range(3):
                            r0, nr = 0, 8
                            if j == 0 and dy == 0:
                                r0, nr = 1, 7
                            if j == 3 and dy == 2:
                                r0, nr = 0, 7
                            soff = chk * XROW + base \
                                + (8 * j + dy + r0 - 1) * (32 * G)
                            for dx in range(3):
                                p0 = 32 * j + 9 * dy + 3 * dx
                                run = 32 * G if dx == 1 else 31 * G
                                pv = tgt[p0:p0 + 3]
                                dst = bass.AP(
                                    tensor=pv.tensor,
                                    offset=pv.offset + r0 * 32 * G
                                    + (G if dx == 0 else 0),
                                    ap=[list(pv.ap[0]), [32 * G, nr],
                                        [1, run]])
                                src = bass.AP(
                                    tensor=d_xq,
                                    offset=soff + (G if dx == 2 else 0),
                                    ap=[[1024 * G, 3], [32 * G, nr],
                                        [1, run]])
                                dma_engs[qi % 3].dma_start(dst, src)
                                qi += 1

                # ---- nibble unpack + int20 decode + bf16 hi/lo split ----
                for n in range(4):
                    nc.vector.tensor_scalar(
                        ap_of(qmu_sb, [[16, 256], [4, 4]], n), qm[:],
                        4 * n, 15, ALU.logical_shift_right, ALU.bitwise_and)
                nc.vector.tensor_scalar(tf_sb[:], qh[:], C_HI, None, ALU.mult)
                nc.vector.tensor_scalar(tm_sb[:], qmu_sb[:], C_LO, -6.0,
                                        ALU.mult, ALU.add)
                nc.vector.tensor_add(tf_sb[:], tf_sb[:], tm_sb[:])
                nc.vector.tensor_copy(ich[:], tf_sb[:])
                nc.vector.tensor_sub(tm_sb[:], tf_sb[:], ich[:])
                nc.vector.tensor_copy(icl[:], tm_sb[:])

                # ---- conv1: 4 psum tiles; tile tau = pooled row pair ----
                # psum free = dr*512 + w*16 + s
                for tau in range(4):
                    p1 = pp1.tile([128, 1024], F32, tag="c1")
                    for dr in range(2):
                        sl = bass.ds((2 * tau + dr) * 512, 512)
                        nc.tensor.matmul(p1[:, dr * 512:(dr + 1) * 512],
                                         w1v, ich[:, sl],
                                         start=True, stop=False)
                        nc.tensor.matmul(p1[:, dr * 512:(dr + 1) * 512],
                                         w1v, icl[:, sl],
                                         start=False, stop=True)
                    # DVE: 2x2 maxpool in ONE XY-window reduce from psum
                    # out free = 16*s + wp
                    pl1 = wk.tile([128, 256], F32, tag="pl1")
                    nc.vector.tensor_reduce(
                        ap_of(pl1, [[16, 16], [1, 16]]),
                        ap_of(p1, [[1, 16], [32, 16], [512, 2], [16, 2]]),
                        mybir.AxisListType.XY, ALU.max)
                    # ACT: sign(bn1) -> +-1 fp8 straight into sp
                    # sp free = rr*288 + s*18 + (wp+1), rr = tau
                    nc.scalar.activation(
                        ap_of(sp, [[18, 16], [1, 16]], tau * 288 + 1),
                        pl1[:], AF.Sign,
                        bias=cst_sb[:, 1:2], scale=cst_sb[:, 0:1])

                # ---- ic2 build: 3 contiguous-run DMAs (SP, ACT, Pool) ----
                RUN = 4 * CH * 18
                for dx in range(3):
                    src = bass.AP(tensor=sp.tensor, offset=sp.offset + dx,
                                  ap=[list(sp.ap[0]), [1, RUN]])
                    dst_t = ic2[32 * dx:32 * (dx + 1)]
                    dst = bass.AP(tensor=dst_t.tensor,
                                  offset=dst_t.offset + CH * 18,
                                  ap=[list(dst_t.ap[0]), [RUN, 4], [1, RUN]])
                    eng = (nc.sync, nc.scalar, nc.gpsimd)[dx]
                    eng.dma_start(dst, src)

            def conv2_part(chk):
                ic2 = ic2_t[chk % NBUF]
                # ---- conv2: 4 one-bank col-packed psum tiles (tau, h) ----
                for tau in range(2):
                    for h in range(2):
                        p2 = pp2.tile([128, 512], F32, tag="c2")
                        for g in range(2):
                            tp = (0, 64 * g) if g else None
                            for dy in range(3):
                                s0 = 8 * tau + 4 * g + 2 * h
                                mv = bass.AP(
                                    tensor=ic2.tensor,
                                    offset=ic2.offset + s0 * 18
                                    + dy * (CH * 18),
                                    ap=[list(ic2.ap[0]), [18, 2],
                                        [CH * 18, 16], [1, 16]])
                                nc.tensor.matmul(
                                    p2[64 * g:64 * (g + 1), :],
                                    w2_sb[:, dy, :], mv,
                                    start=(dy == 0), stop=(dy == 2),
                                    tile_position=tp)
                        # DVE: 2x2 maxpool, one XY-window reduce
                        xm2b = wk.tile([128, 128], F32, tag="xm2b")
                        nc.vector.tensor_reduce(
                            ap_of(xm2b, [[8, 16], [1, 8]]),
                            ap_of(p2, [[32, 16], [2, 8], [16, 2], [1, 2]]),
                            mybir.AxisListType.XY, ALU.max)
                        # ACT sign(bn2) -> +-1 fp8 into s2all
                        sh0 = (2 * chk + tau) * 4 + 2 * h
                        nc.scalar.activation(
                            ap_of(s2all, [[1, 2], [1024, 8], [128, 8]], sh0),
                            xm2b[:], AF.Sign,
                            bias=cst_sb[:, 3:4], scale=cst_sb[:, 2:3])

                # ---- repack waves: one DMA per (g, yxp, wave) ----
                # waves at chk 7 (sh 0:64), 11 (64:96), 15 (96:128) so the
                # final fc1 dependency is only a quarter-size transfer
                WAVES = {7: (0, 64), 11: (64, 32), 15: (96, 32)}
                if chk in WAVES:
                    lo, ln = WAVES[chk]
                    for g in range(2):
                        for yxp in range(2):
                            src_t = s2all[64 * g:64 * (g + 1)]
                            src = bass.AP(
                                tensor=src_t.tensor,
                                offset=src_t.offset + 128 * yxp + lo,
                                ap=[list(src_t.ap[0]), [256, 32], [1, ln]])
                            dst_t = fc1in[64 * yxp:64 * (yxp + 1)]
                            dst = bass.AP(
                                tensor=dst_t.tensor,
                                offset=dst_t.offset + g * 128 + lo,
                                ap=[list(dst_t.ap[0]), [256, 32], [1, ln]])
                            eng = (nc.sync, nc.scalar, nc.gpsimd,
                                   nc.gpsimd)[2 * g + yxp]
                            eng.dma_start(dst, src)

            conv1_part(0)
            for chk in range(1, NCHUNK):
                conv1_part(chk)
                conv2_part(chk - 1)
            conv2_part(NCHUNK - 1)

            # ================= fc phase =================
            # fc1: 4 m-tiles, fp8 DoubleRow over kt pairs, column-sliced:
            # cols filled by waves 1+2 (sh 0:96 of both g-blocks) run while
            # wave 3 is still transferring; the wave-3 cols follow.
            pfs = []
            for m in range(4):
                pf = (pp1 if m < 3 else pp2).tile(
                    [128, 1024] if m < 3 else [128, 512], F32,
                    tag="c1" if m < 3 else "c2", name=f"pf_{m}")
                pfs.append(pf)
            SLICES = [[(0, 96), (128, 96)], [(96, 32), (224, 32)]]
            for phase in range(2):
                for m in range(4):
                    pf = pfs[m]
                    for c0, wdt in SLICES[phase]:
                        for kt in range(16):
                            lw = bass.AP(
                                tensor=fc1w_sb.tensor,
                                offset=fc1w_sb.offset + 2 * kt * 512
                                + 128 * m,
                                ap=[list(fc1w_sb.ap[0]), [512, 2], [1, 128]])
                            mv = bass.AP(
                                tensor=fc1in.tensor,
                                offset=fc1in.offset + 2 * kt * BC + c0,
                                ap=[list(fc1in.ap[0]), [BC, 2], [1, wdt]])
                            nc.tensor.matmul(
                                pf[:, c0:c0 + wdt], lw, mv,
                                start=(kt == 0), stop=(kt == 15),
                                perf_mode=mybir.MatmulPerfMode.DoubleRow)
            for m in range(4):
                nc.scalar.activation(s3_sb[:, m, :], pfs[m][:, :BC], AF.Sign,
                                     bias=cst_sb[:, 8 + m:9 + m],
                                     scale=cst_sb[:, 4 + m:5 + m])

            # fc2: 2 m-tiles, fp8 DoubleRow over kt pairs; clip + hi/lo
            # split per m2-half so the DVE chain overlaps fc2's second half
            for m2 in range(2):
                pg = pp1.tile([128, 1024], F32, tag="c1")
                for kt in range(2):
                    lw = bass.AP(
                        tensor=fc2w_sb.tensor,
                        offset=fc2w_sb.offset + 2 * kt * 256 + 128 * m2,
                        ap=[list(fc2w_sb.ap[0]), [256, 2], [1, 128]])
                    mv = bass.AP(
                        tensor=s3_sb.tensor,
                        offset=s3_sb.offset + 2 * kt * BC,
                        ap=[list(s3_sb.ap[0]), [BC, 2], [1, BC]])
                    nc.tensor.matmul(pg[:, :BC], lw, mv,
                                     start=(kt == 0), stop=(kt == 1),
                                     perf_mode=mybir.MatmulPerfMode.DoubleRow)
                nc.scalar.activation(u4_sb[:, m2, :], pg[:, :BC], AF.Identity,
                                     bias=cst_sb[:, 14 + m2:15 + m2],
                                     scale=cst_sb[:, 12 + m2:13 + m2])
                nc.vector.tensor_scalar(u4_sb[:, m2, :], u4_sb[:, m2, :],
                                        1.0, -1.0, ALU.min, ALU.max)
                nc.vector.tensor_copy(s4h_sb[:, m2, :], u4_sb[:, m2, :])
                nc.vector.tensor_sub(s4r_sb[:, m2, :], u4_sb[:, m2, :],
                                     s4h_sb[:, m2, :])
                nc.vector.tensor_copy(s4l_sb[:, m2, :], s4r_sb[:, m2, :])

            # fc3 + log_softmax; batch tile bt == g block of fc1 columns.
            h3s, mxs, negs, ses, lss = [], [], [], [], []
            for bt in range(2):
                ph = pp2.tile([128, 512], F32, tag="c2")
                mms = []
                for kt in range(2):
                    lh = s4h_sb[:, kt, 128 * bt:128 * (bt + 1)]
                    ll = s4l_sb[:, kt, 128 * bt:128 * (bt + 1)]
                    w3hv = sbf_sb[:, 128 + 10 * kt:138 + 10 * kt]
                    w3lv = sbf_sb[:, 148 + 10 * kt:158 + 10 * kt]
                    mms += [(lh, w3hv), (ll, w3hv), (lh, w3lv)]
                for i, (lhs, rhs) in enumerate(mms):
                    nc.tensor.matmul(ph[:, :10], lhs, rhs,
                                     start=(i == 0), stop=(i == len(mms) - 1))
                h3 = wk.tile([128, 10], F32, tag="h3", name=f"h3_{bt}")
                nc.vector.tensor_add(h3[:], ph[:, :10], cst_sb[:, 16:26])
                mx = wk.tile([128, 1], F32, tag="mx", name=f"mx_{bt}")
                nc.vector.tensor_reduce(mx[:], h3[:], mybir.AxisListType.X,
                                        ALU.max)
                negmx = wk.tile([128, 1], F32, tag="negmx", name=f"negmx_{bt}")
                nc.vector.tensor_scalar_mul(negmx[:], mx[:], -1.0)
                h3s.append(h3)
                mxs.append(mx)
                negs.append(negmx)
            for bt in range(2):     # both Exps together (one act table set)
                et = wk.tile([128, 10], F32, tag="et", name=f"et_{bt}")
                se = wk.tile([128, 1], F32, tag="se", name=f"se_{bt}")
                nc.scalar.activation(et[:], h3s[bt][:], AF.Exp,
                                     bias=negs[bt][:], scale=1.0,
                                     accum_out=se[:])
                ses.append(se)
            for bt in range(2):     # then both Lns (single table reload)
                ls = wk.tile([128, 1], F32, tag="ls", name=f"ls_{bt}")
                nc.scalar.activation(ls[:], ses[bt][:], AF.Ln)
                lss.append(ls)
            for bt in range(2):
                tt = wk.tile([128, 1], F32, tag="tt", name=f"tt_{bt}")
                nc.vector.tensor_add(tt[:], mxs[bt][:], lss[bt][:])
                o = wk.tile([128, 10], F32, tag="o", name=f"o_{bt}")
                nc.vector.tensor_scalar_sub(o[:], h3s[bt][:], tt[:])
                # un-permute: psum partition p -> sample row
                # s = 8*(p//4) + 4*bt + p%4
                dst = bass.AP(tensor=d_out, offset=bt * 40,
                              ap=[[80, 32], [10, 4], [1, 10]])
                (nc.sync if bt == 0 else nc.scalar).dma_start(dst, o[:])

    nc.compile()
    return nc


def _get_module():
    if "nc" not in _nc_cache:
        nc = _build_module()
        # the module is frozen after compile; memoize its (identical)
        # serialization so per-call re-jits don't re-serialize ~30ms worth
        js = nc.to_json_bytes()
        nc.to_json_bytes = lambda: js
        _nc_cache["nc"] = nc
    return _nc_cache["nc"]


def kernel(**inputs):
    from concourse.bass_utils import run_bass_kernel_spmd

    in_maps = _host_prep(inputs)
    nc = _get_module()
    res = run_bass_kernel_spmd(nc, in_maps, core_ids=list(range(NCORES)))
    out = np.concatenate([r["out"] for r in res.results], axis=0)
    return out.astype(np.float32)


# revision 46
# speedup vs baseline: 1.3003x; 1.0132x over previous
"""BinarizedLeNet5/CIFAR10 Trainium2 kernel (8-core data parallel), v4.

The graded metric is wall-clock of a warm run_bass_kernel_spmd call, and
the axon tunnel moves ~50 MB/s — so v2's 287 MB of host-staged im2col
inputs WAS the runtime.  v4 ships ~16.5 MB instead:

- x goes up as int20 fixed-point over [-6,6): u16 hi + nibble-packed u4
  mantissa extension, unpadded, samples innermost.  The im2col expansion
  happens on-device with 36 strided gather DMAs per chunk-half (3-dim
  APs, contiguous (col,s) runs); halo positions keep a one-time q0
  memset that decodes to exact 0.  DVE unpacks the nibbles, decodes to
  f32 and splits into the bf16 hi/lo pair the conv1 matmuls consume
  (the split residual ~x*2^-18 dominates the encoding error; rel err
  1.12e-2 vs the 2e-2 gate).  conv1 psum free layout is (r, w, s).
- fc1w+fc2w ship bit-packed in one [128,2176] u8 tensor, unpacked
  on-device with DVE shift/and ops.
- all weight tensors arrive 8-way column-sharded and are AllGather'd
  on-device (replicated upload would cost 8x the bytes).
- small f32 constants consolidate into one [128,32] tensor, w1/w3h/w3l
  into one [128,168] bf16 tensor: 6 input args total.

Everything downstream of conv1's ACT-sign (sp/ic2 build, conv2, fc
phase, log_softmax tail) is byte-identical to v2.  A persistent XLA
compilation cache is configured at import so warm calls skip the
~0.5s client-side BIR re-verification that run_bass_kernel_spmd's
per-call re-jit otherwise pays.
"""
import sys
import numpy as np

sys.path.insert(0, "/opt/pypackages")
sys.path.insert(0, "/opt/trn_rl_repo")

import ml_dtypes

# Persistent XLA compilation cache: run_bass_kernel_spmd re-jits on every
# call (fresh closure), and without this each warm call pays ~0.5s of
# client-side BIR re-verification + DVE table gen before the NEFF cache
# hits.  The persistent cache keys on HLO fingerprint, so call 2+ skips
# backend compile entirely.
import jax

jax.config.update("jax_compilation_cache_dir", "/tmp/jax_comp_cache")
jax.config.update("jax_persistent_cache_min_entry_size_bytes", -1)
jax.config.update("jax_persistent_cache_min_compile_time_secs", 0.0)

BF = ml_dtypes.bfloat16
F8 = ml_dtypes.float8_e4m3
NCORES = 8
B = 2048
BC = B // NCORES          # 256 samples per core
CH = 16                   # samples per chunk
NCHUNK = BC // CH         # 16 chunks
EPS = np.float32(1e-5)
XSPLIT = 4                # x upload streams (arg count tradeoff)
XCHW = 3 * 32 * 32 * CH   # chunk elements, unpadded (49152)
XCHM = XCHW // 2          # nibble-packed chunk bytes (24576)
XSC = float(2 ** 20) / 12.0   # int20 fixed-point scale over [-6, 6)
Q0 = 1 << 19                  # encoding of x == 0.0
C_HI = 3.0 * 2.0 ** -14       # decode: x = hi16*C_HI + m4*C_LO - 6
C_LO = 3.0 * 2.0 ** -18

_nc_cache = {}


def _f32(x):
    return np.asarray(x, np.float32)


def _host_prep(inputs):
    """Build all per-core device input arrays (layout prep only)."""
    x = _f32(inputs["x"])                      # [2048,3,32,32]

    # ---- x as int20 fixed-point over [-6,6): u16 hi + u4 nibble ext,
    # unpadded, samples innermost (halo positions stay at the q0 memset) ----
    q = np.clip(np.round((x.astype(np.float64) + 6.0) * XSC),
                0, 2 ** 20 - 1).astype(np.uint32)

    def stage(xq):
        xt = xq.reshape(NCORES, NCHUNK, CH, 3, 32, 32)
        xt = np.ascontiguousarray(xt.transpose(0, 1, 3, 4, 5, 2))
        return xt.reshape(NCORES, NCHUNK, -1)

    xh = stage((q >> 4).astype(np.uint16))
    m4 = stage((q & 15).astype(np.uint8))       # [.., pos, s]
    m4 = m4.reshape(NCORES, NCHUNK, XCHM, 2)    # byte = (s even, s odd)
    xl = (m4[..., 0] | (m4[..., 1] << 4)).astype(np.uint8)
    # one u16 arg per core: [hi16 | nibble bytes viewed as u16 pairs]
    xq = np.concatenate(
        [xh, np.ascontiguousarray(xl).view(np.uint16)], axis=2)

    # ---- conv1 stationary: block-diag, k = 3*(3dy+dx)+c, out p = 4*co+j ----
    w1s = np.sign(_f32(inputs["conv1_w"]))               # [32,3,3,3]
    w1k = np.ascontiguousarray(w1s.transpose(2, 3, 1, 0)).reshape(27, 32)
    # bf16 blob: [:,0:128]=w1, [:,128:148]=w3h (kt-major), [:,148:168]=w3l
    wbf = np.zeros((128, 168), BF)
    for j in range(4):
        for co in range(32):
            wbf[32 * j:32 * j + 27, 4 * co + j] = w1k[:, co].astype(BF)
    w3 = _f32(inputs["fc3_w"]).T                         # [256,10]
    w3h = w3.astype(BF)
    w3l = (w3 - w3h.astype(np.float32)).astype(BF)
    for kt in range(2):
        wbf[:, 128 + 10 * kt:138 + 10 * kt] = w3h[128 * kt:128 * (kt + 1)]
        wbf[:, 148 + 10 * kt:158 + 10 * kt] = w3l[128 * kt:128 * (kt + 1)]

    # ---- conv2 stationaries [3][96,64]: p = 32*dx + c (bit-packed) ----
    w2s = np.sign(_f32(inputs["conv2_w"]))               # [64,32,3,3]
    w2_st = np.zeros((3, 96, 64), np.float32)
    for dy in range(3):
        for dx in range(3):
            w2_st[dy, 32 * dx:32 * dx + 32] = w2s[:, :, dy, dx].T

    # ---- consolidated f32 constants [128, 32] (padded for 8-way shard) ----
    cst = np.zeros((128, 32), np.float32)
    inv1 = _f32(inputs["bn1_g"]) / np.sqrt(_f32(inputs["bn1_v"]) + EPS)
    sh1c = (_f32(inputs["conv1_b"]) - _f32(inputs["bn1_m"])) * inv1 \
        + _f32(inputs["bn1_b"])
    cst[:, 0] = np.repeat(inv1, 4)
    cst[:, 1] = np.repeat(sh1c, 4)
    inv2 = _f32(inputs["bn2_g"]) / np.sqrt(_f32(inputs["bn2_v"]) + EPS)
    sh2c = _f32(inputs["bn2_b"]) - _f32(inputs["bn2_m"]) * inv2
    cst[:, 2] = np.tile(inv2, 2)
    cst[:, 3] = np.tile(_f32(inputs["conv2_b"]) * inv2 + sh2c, 2)
    inv3 = _f32(inputs["bn3_g"]) / np.sqrt(_f32(inputs["bn3_v"]) + EPS)
    sh3c = (_f32(inputs["fc1_b"]) - _f32(inputs["bn3_m"])) * inv3 \
        + _f32(inputs["bn3_b"])
    cst[:, 4:8] = inv3.reshape(4, 128).T
    cst[:, 8:12] = sh3c.reshape(4, 128).T
    inv4 = _f32(inputs["bn4_g"]) / np.sqrt(_f32(inputs["bn4_v"]) + EPS)
    sh4c = (_f32(inputs["fc2_b"]) - _f32(inputs["bn4_m"])) * inv4 \
        + _f32(inputs["bn4_b"])
    cst[:, 12:14] = inv4.reshape(2, 128).T
    cst[:, 14:16] = sh4c.reshape(2, 128).T
    cst[:, 16:26] = _f32(inputs["fc3_b"])[None, :]

    # ---- fc1+fc2+w2 bit-packed into one [128, 2200] u8 tensor ----
    # fc1: kt = yx//2, kp = c + 64*(yx%2), bits along m (cols 0:2048);
    # fc2 at 2048:2176; w2 (rows 0:96 only) at 2176:2200
    fw1 = np.sign(_f32(inputs["fc1_w"]))                 # [512,4096]
    A = fw1.T.reshape(64, 64, 512)                       # [c][yx][m]
    Bm = A.reshape(64, 32, 2, 512)                       # [c][kt][yxp][m]
    fc1_st = np.ascontiguousarray(
        Bm.transpose(1, 2, 0, 3).reshape(32, 128, 512))  # [kt][kp][m]
    fw2 = np.sign(_f32(inputs["fc2_w"]))                 # [256,512]
    fc2_st = np.ascontiguousarray(fw2.T.reshape(4, 128, 256))
    w2v = np.zeros((128, 192), np.float32)               # [p][dy*64+m]
    w2v[:96] = np.ascontiguousarray(
        w2_st.transpose(1, 0, 2)).reshape(96, 192)

    wpk = np.zeros((128, 2200), np.uint8)
    for st, off in ((fc1_st.transpose(1, 0, 2).reshape(128, -1), 0),
                    (fc2_st.transpose(1, 0, 2).reshape(128, -1), 2048),
                    (w2v, 2176)):
        bits = (st > 0).astype(np.uint8)
        for b in range(8):
            wpk[:, off:off + st.shape[1] // 8] |= (bits[:, b::8]
                                                   << np.uint8(b))

    # cst ships as a bf16 hi/lo pair alongside wbf (one bf16 arg)
    ch = cst.astype(BF)
    cl = (cst - ch.astype(np.float32)).astype(BF)
    sbf = np.concatenate([wbf, ch, cl], axis=1)          # [128, 232]

    # weights ship 8-way column-sharded and AllGather on device; x ships
    # split into XSPLIT args (a few parallel tunnel streams beat one)
    PER = NCHUNK // XSPLIT
    in_maps = []
    for ci in range(NCORES):
        m = {
            "s8": np.ascontiguousarray(wpk[:, 275 * ci:275 * (ci + 1)]),
            "sbf": np.ascontiguousarray(sbf[:, 29 * ci:29 * (ci + 1)]),
        }
        for k in range(XSPLIT):
            m[f"xq{k}"] = np.ascontiguousarray(xq[ci, k * PER:(k + 1) * PER])
        in_maps.append(m)
    return in_maps


def _build_module(reps=1):
    import concourse.bass as bass
    import concourse.mybir as mybir
    import concourse.tile as tile
    from concourse import bacc
    from contextlib import ExitStack

    F32 = mybir.dt.float32
    BF16 = mybir.dt.bfloat16
    FP8 = mybir.dt.float8e4
    U8 = mybir.dt.uint8
    AF = mybir.ActivationFunctionType
    ALU = mybir.AluOpType

    nc = bacc.Bacc("TRN2", target_bir_lowering=False, debug=False,
                   num_devices=NCORES)

    U16 = mybir.dt.uint16
    RG = [list(range(NCORES))]

    # ---- DRAM tensors: weights arrive 8-way sharded, AllGather on-device;
    # x is one u16 tensor per core: [hi16 | nibble-pairs] per chunk row ----
    XROW = XCHW + XCHM // 2
    d_xq = [nc.dram_tensor(f"xq{k}", [NCHUNK // XSPLIT, XROW], U16,
                           kind="ExternalInput") for k in range(XSPLIT)]
    shards = {}
    for nm, shp, dt in (("s8", [128, 275], U8), ("sbf", [128, 29], BF16)):
        d_s = nc.dram_tensor(nm, shp, dt, kind="ExternalInput")
        d_i = nc.dram_tensor("i" + nm, shp, dt, kind="Internal")
        d_g = nc.dram_tensor("g" + nm, [NCORES] + shp, dt, kind="Internal",
                             addr_space="Shared")
        shards[nm] = (d_s, d_i, d_g)
    d_out = nc.dram_tensor("out", [BC, 10], F32, kind="ExternalOutput")

    NBUF = 3                       # sp/ic2 ping-pong depth
    NXB = 2                        # ic1 staging ping-pong depth
    SPW = 4 * CH * 18              # sp payload per partition
    SP_FREE = SPW + 8
    IC2_FREE = 18 * CH * 18 + 8    # (R 18, s CH, W 18)

    with tile.TileContext(nc) as tc, ExitStack() as ctx:
        const = ctx.enter_context(tc.tile_pool(name="const", bufs=1))
        wk = ctx.enter_context(tc.tile_pool(name="wk", bufs=3))
        pp1 = ctx.enter_context(tc.tile_pool(name="pp1", bufs=3, space="PSUM"))
        pp2 = ctx.enter_context(tc.tile_pool(name="pp2", bufs=2, space="PSUM"))

        # ---- persistent tiles ----
        sbf_sb = const.tile([128, 232], BF16, tag="sbf")
        w2_sb = const.tile([96, 3, 64], FP8, tag="w2")
        fc1w_sb = const.tile([128, 32, 512], FP8, tag="fc1w")
        fc2w_sb = const.tile([128, 4, 256], FP8, tag="fc2w")
        cst_sb = const.tile([128, 32], F32, tag="cst")
        pk_sb = const.tile([128, 2200], U8, tag="pk")
        tmp_sb = const.tile([128, 2200], U8, tag="tmp")
        qh_t = [const.tile([128, 4096], U16, tag=f"qh{i}", name=f"qh{i}")
                for i in range(NXB)]
        qm_t = [const.tile([128, 1024], U16, tag=f"qm{i}", name=f"qm{i}")
                for i in range(NXB)]
        qmu_sb = const.tile([128, 4096], U16, tag="qmu")
        ich_t = [const.tile([128, 4096], BF16, tag=f"ich{i}", name=f"ich{i}")
                 for i in range(NXB)]
        icl_t = [const.tile([128, 4096], BF16, tag=f"icl{i}", name=f"icl{i}")
                 for i in range(NXB)]
        tf_sb = const.tile([128, 4096], F32, tag="tf")
        tm_sb = const.tile([128, 4096], F32, tag="tm")
        sp_t = [const.tile([128, SP_FREE], FP8, tag=f"sp{i}", name=f"sp{i}")
                for i in range(NBUF)]
        ic2_t = [const.tile([96, IC2_FREE], FP8, tag=f"ic2_{i}", name=f"ic2_{i}")
                 for i in range(NBUF)]
        # s2all: partition (g2, c64), free = yx*128 + sh,
        #   sh = (2*chk + tau)*4 + sl,  sample s = 8*(sh//4) + 4g + sh%4
        s2all = const.tile([128, 64 * 128], FP8, tag="s2all")
        # fc1in: partition kp = c + 64*(yx%2), free = kt*BC + (g*128 + sh)
        fc1in = const.tile([128, 32 * BC], FP8, tag="fc1in")
        s3_sb = const.tile([128, 4, BC], FP8, tag="s3")
        u4_sb = const.tile([128, 2, BC], F32, tag="u4")
        s4h_sb = const.tile([128, 2, BC], BF16, tag="s4h")
        s4l_sb = const.tile([128, 2, BC], BF16, tag="s4l")
        s4r_sb = const.tile([128, 2, BC], F32, tag="s4r")

        w1v = sbf_sb[:, 0:128]

        def ap_of(t, dims, off=0):
            return bass.AP(tensor=t.tensor, offset=t.offset + off,
                           ap=[list(t.ap[0])] + [list(d) for d in dims])

        # ---- setup: AllGather the weight shards, then load to SBUF ----
        for nm, (d_s, d_i, d_g) in shards.items():
            nc.gpsimd.dma_start(d_i.ap(), d_s.ap())
            nc.gpsimd.collective_compute(
                "AllGather", ALU.bypass, RG, ins=[d_i.ap()], outs=[d_g.ap()])
        for c in range(NCORES):
            g = shards["s8"][2].ap()[c]
            nc.gpsimd.dma_start(pk_sb[:, 275 * c:275 * (c + 1)], g)
            g = shards["sbf"][2].ap()[c]
            nc.gpsimd.dma_start(sbf_sb[:, 29 * c:29 * (c + 1)], g)
        # cst = bf16 hi + bf16 lo
        nc.vector.tensor_add(cst_sb[:], sbf_sb[:, 168:200],
                             sbf_sb[:, 200:232])

        # fc1w/fc2w/w2 unpack: bit b of packed byte mb -> m = 8*mb + b
        for b in range(8):
            nc.vector.tensor_scalar(tmp_sb[:], pk_sb[:], b, 1,
                                    ALU.logical_shift_right, ALU.bitwise_and)
            dst = bass.AP(tensor=fc1w_sb.tensor, offset=fc1w_sb.offset + b,
                          ap=[list(fc1w_sb.ap[0]), [512, 32], [8, 64]])
            nc.vector.tensor_scalar(dst, bass.AP(
                tensor=tmp_sb.tensor, offset=tmp_sb.offset,
                ap=[list(tmp_sb.ap[0]), [1, 2048]]), 2.0, -1.0,
                ALU.mult, ALU.add)
            dst2 = bass.AP(tensor=fc2w_sb.tensor, offset=fc2w_sb.offset + b,
                           ap=[list(fc2w_sb.ap[0]), [256, 4], [8, 32]])
            nc.vector.tensor_scalar(dst2, bass.AP(
                tensor=tmp_sb.tensor, offset=tmp_sb.offset + 2048,
                ap=[list(tmp_sb.ap[0]), [1, 128]]), 2.0, -1.0,
                ALU.mult, ALU.add)
            dst3 = bass.AP(tensor=w2_sb.tensor, offset=w2_sb.offset + b,
                           ap=[list(w2_sb.ap[0]), [8, 24]])
            tv = tmp_sb[0:96]
            nc.vector.tensor_scalar(dst3, bass.AP(
                tensor=tv.tensor, offset=tv.offset + 2176,
                ap=[list(tv.ap[0]), [1, 24]]), 2.0, -1.0,
                ALU.mult, ALU.add)

        # init the gather tiles to the encoding of 0.0 once: pad partitions
        # and clipped row slabs stay at q0 forever and decode to exact 0.
        for t in qh_t:
            nc.vector.memset(t[:], Q0 >> 4)
        for t in qm_t:
            nc.vector.memset(t[:], 0)
        # sp pads: cols w==0 and w==17 of each 18-block, plus the 8 slack cols
        for t in sp_t:
            nc.vector.memset(
                ap_of(t, [[18, SPW // 18], [17, 2], [1, 1]]), 0.0)
            nc.vector.memset(ap_of(t, [[1, 8]], SPW), 0.0)
        # ic2 pads: halo rows R=0 and R=17 (+ slack)
        for t in ic2_t:
            nc.vector.memset(
                ap_of(t, [[17 * CH * 18, 2], [1, CH * 18]]), 0.0)
            nc.vector.memset(ap_of(t, [[1, 8]], 18 * CH * 18), 0.0)

        dma_engs = [nc.sync, nc.scalar, nc.gpsimd]

        for _rep in range(reps):
            # ================= chunk loop (software-pipelined) =================
            # iteration k emits conv1 of chunk k, then conv2 of chunk k-1, so
            # the in-order PE queue never stalls on chunk k-1's ic2 DMAs.
            def conv1_part(chk):
                sp = sp_t[chk % NBUF]
                ic2 = ic2_t[chk % NBUF]
                qh = qh_t[chk % NXB]
                qm = qm_t[chk % NXB]
                ich = ich_t[chk % NXB]
                icl = icl_t[chk % NXB]

                # ---- on-device im2col: 36 gather DMAs per half ----
                # dst w-range / src col-range clip at the image edge; the
                # unwritten halo positions keep their q0 memset (decode 0).
                qi = 0
                for tgt, base, G in ((qh, 0, 16), (qm, XCHW, 4)):
                    for j in range(4):
                        for dy in range(3):
                            r0, nr = 0, 8
                            if j == 0 and dy == 0:
                                r0, nr = 1, 7
                            if j == 3 and dy == 2:
                                r0, nr = 0, 7
                            PER = NCHUNK // XSPLIT
                            dxt = d_xq[chk // PER]
                            soff = (chk % PER) * XROW + base \
                                + (8 * j + dy + r0 - 1) * (32 * G)
                            for dx in range(3):
                                p0 = 32 * j + 9 * dy + 3 * dx
                                run = 32 * G if dx == 1 else 31 * G
                                pv = tgt[p0:p0 + 3]
                                dst = bass.AP(
                                    tensor=pv.tensor,
                                    offset=pv.offset + r0 * 32 * G
                                    + (G if dx == 0 else 0),
                                    ap=[list(pv.ap[0]), [32 * G, nr],
                                        [1, run]])
                                src = bass.AP(
                                    tensor=dxt,
                                    offset=soff + (G if dx == 2 else 0),
                                    ap=[[1024 * G, 3], [32 * G, nr],
                                        [1, run]])
                                dma_engs[qi % 3].dma_start(dst, src)
                                qi += 1

                # ---- nibble unpack + int20 decode + bf16 hi/lo split ----
                for n in range(4):
                    nc.vector.tensor_scalar(
                        ap_of(qmu_sb, [[16, 256], [4, 4]], n), qm[:],
                        4 * n, 15, ALU.logical_shift_right, ALU.bitwise_and)
                nc.vector.tensor_scalar(tf_sb[:], qh[:], C_HI, None, ALU.mult)
                nc.vector.tensor_scalar(tm_sb[:], qmu_sb[:], C_LO, -6.0,
                                        ALU.mult, ALU.add)
                nc.vector.tensor_add(tf_sb[:], tf_sb[:], tm_sb[:])
                nc.vector.tensor_copy(ich[:], tf_sb[:])
                nc.vector.tensor_sub(tm_sb[:], tf_sb[:], ich[:])
                nc.vector.tensor_copy(icl[:], tm_sb[:])

                # ---- conv1: 4 psum tiles; tile tau = pooled row pair ----
                # psum free = dr*512 + w*16 + s
                for tau in range(4):
                    p1 = pp1.tile([128, 1024], F32, tag="c1")
                    for dr in range(2):
                        sl = bass.ds((2 * tau + dr) * 512, 512)
                        nc.tensor.matmul(p1[:, dr * 512:(dr + 1) * 512],
                                         w1v, ich[:, sl],
                                         start=True, stop=False)
                        nc.tensor.matmul(p1[:, dr * 512:(dr + 1) * 512],
                                         w1v, icl[:, sl],
                                         start=False, stop=True)
                    # DVE: 2x2 maxpool in ONE XY-window reduce from psum
                    # out free = 16*s + wp
                    pl1 = wk.tile([128, 256], F32, tag="pl1")
                    nc.vector.tensor_reduce(
                        ap_of(pl1, [[16, 16], [1, 16]]),
                        ap_of(p1, [[1, 16], [32, 16], [512, 2], [16, 2]]),
                        mybir.AxisListType.XY, ALU.max)
                    # ACT: sign(bn1) -> +-1 fp8 straight into sp
                    # sp free = rr*288 + s*18 + (wp+1), rr = tau
                    nc.scalar.activation(
                        ap_of(sp, [[18, 16], [1, 16]], tau * 288 + 1),
                        pl1[:], AF.Sign,
                        bias=cst_sb[:, 1:2], scale=cst_sb[:, 0:1])

                # ---- ic2 build: 3 contiguous-run DMAs (SP, ACT, Pool) ----
                RUN = 4 * CH * 18
                for dx in range(3):
                    src = bass.AP(tensor=sp.tensor, offset=sp.offset + dx,
                                  ap=[list(sp.ap[0]), [1, RUN]])
                    dst_t = ic2[32 * dx:32 * (dx + 1)]
                    dst = bass.AP(tensor=dst_t.tensor,
                                  offset=dst_t.offset + CH * 18,
                                  ap=[list(dst_t.ap[0]), [RUN, 4], [1, RUN]])
                    eng = (nc.sync, nc.scalar, nc.gpsimd)[dx]
                    eng.dma_start(dst, src)

            def conv2_part(chk):
                ic2 = ic2_t[chk % NBUF]
                # ---- conv2: 4 one-bank col-packed psum tiles (tau, h) ----
                for tau in range(2):
                    for h in range(2):
                        p2 = pp2.tile([128, 512], F32, tag="c2")
                        for g in range(2):
                            tp = (0, 64 * g) if g else None
                            for dy in range(3):
                                s0 = 8 * tau + 4 * g + 2 * h
                                mv = bass.AP(
                                    tensor=ic2.tensor,
                                    offset=ic2.offset + s0 * 18
                                    + dy * (CH * 18),
                                    ap=[list(ic2.ap[0]), [18, 2],
                                        [CH * 18, 16], [1, 16]])
                                nc.tensor.matmul(
                                    p2[64 * g:64 * (g + 1), :],
                                    w2_sb[:, dy, :], mv,
                                    start=(dy == 0), stop=(dy == 2),
                                    tile_position=tp)
                        # DVE: 2x2 maxpool, one XY-window reduce
                        xm2b = wk.tile([128, 128], F32, tag="xm2b")
                        nc.vector.tensor_reduce(
                            ap_of(xm2b, [[8, 16], [1, 8]]),
                            ap_of(p2, [[32, 16], [2, 8], [16, 2], [1, 2]]),
                            mybir.AxisListType.XY, ALU.max)
                        # ACT sign(bn2) -> +-1 fp8 into s2all
                        sh0 = (2 * chk + tau) * 4 + 2 * h
                        nc.scalar.activation(
                            ap_of(s2all, [[1, 2], [1024, 8], [128, 8]], sh0),
                            xm2b[:], AF.Sign,
                            bias=cst_sb[:, 3:4], scale=cst_sb[:, 2:3])

                # ---- repack waves: one DMA per (g, yxp, wave) ----
                # waves at chk 7 (sh 0:64), 11 (64:96), 15 (96:128) so the
                # final fc1 dependency is only a quarter-size transfer
                WAVES = {7: (0, 64), 11: (64, 32), 15: (96, 32)}
                if chk in WAVES:
                    lo, ln = WAVES[chk]
                    for g in range(2):
                        for yxp in range(2):
                            src_t = s2all[64 * g:64 * (g + 1)]
                            src = bass.AP(
                                tensor=src_t.tensor,
                                offset=src_t.offset + 128 * yxp + lo,
                                ap=[list(src_t.ap[0]), [256, 32], [1, ln]])
                            dst_t = fc1in[64 * yxp:64 * (yxp + 1)]
                            dst = bass.AP(
                                tensor=dst_t.tensor,
                                offset=dst_t.offset + g * 128 + lo,
                                ap=[list(dst_t.ap[0]), [256, 32], [1, ln]])
                            eng = (nc.sync, nc.scalar, nc.gpsimd,
                                   nc.gpsimd)[2 * g + yxp]
                            eng.dma_start(dst, src)

            conv1_part(0)
            for chk in range(1, NCHUNK):
                conv1_part(chk)
                conv2_part(chk - 1)
            conv2_part(NCHUNK - 1)

            # ================= fc phase =================
            # fc1: 4 m-tiles, fp8 DoubleRow over kt pairs, column-sliced:
            # cols filled by waves 1+2 (sh 0:96 of both g-blocks) run while
            # wave 3 is still transferring; the wave-3 cols follow.
            pfs = []
            for m in range(4):
                pf = (pp1 if m < 3 else pp2).tile(
                    [128, 1024] if m < 3 else [128, 512], F32,
                    tag="c1" if m < 3 else "c2", name=f"pf_{m}")
                pfs.append(pf)
            SLICES = [[(0, 96), (128, 96)], [(96, 32), (224, 32)]]
            for phase in range(2):
                for m in range(4):
                    pf = pfs[m]
                    for c0, wdt in SLICES[phase]:
                        for kt in range(16):
                            lw = bass.AP(
                                tensor=fc1w_sb.tensor,
                                offset=fc1w_sb.offset + 2 * kt * 512
                                + 128 * m,
                                ap=[list(fc1w_sb.ap[0]), [512, 2], [1, 128]])
                            mv = bass.AP(
                                tensor=fc1in.tensor,
                                offset=fc1in.offset + 2 * kt * BC + c0,
                                ap=[list(fc1in.ap[0]), [BC, 2], [1, wdt]])
                            nc.tensor.matmul(
                                pf[:, c0:c0 + wdt], lw, mv,
                                start=(kt == 0), stop=(kt == 15),
                                perf_mode=mybir.MatmulPerfMode.DoubleRow)
            for m in range(4):
                nc.scalar.activation(s3_sb[:, m, :], pfs[m][:, :BC], AF.Sign,
                                     bias=cst_sb[:, 8 + m:9 + m],
                                     scale=cst_sb[:, 4 + m:5 + m])

            # fc2: 2 m-tiles, fp8 DoubleRow over kt pairs; clip + hi/lo
            # split per m2-half so the DVE chain overlaps fc2's second half
            for m2 in range(2):
                pg = pp1.tile([128, 1024], F32, tag="c1")
                for kt in range(2):
                    lw = bass.AP(
                        tensor=fc2w_sb.tensor,
                        offset=fc2w_sb.offset + 2 * kt * 256 + 128 * m2,
                        ap=[list(fc2w_sb.ap[0]), [256, 2], [1, 128]])
                    mv = bass.AP(
                        tensor=s3_sb.tensor,
                        offset=s3_sb.offset + 2 * kt * BC,
                        ap=[list(s3_sb.ap[0]), [BC, 2], [1, BC]])
                    nc.tensor.matmul(pg[:, :BC], lw, mv,
                                     start=(kt == 0), stop=(kt == 1),
                                     perf_mode=mybir.MatmulPerfMode.DoubleRow)
                nc.scalar.activation(u4_sb[:, m2, :], pg[:, :BC], AF.Identity,
                                     bias=cst_sb[:, 14 + m2:15 + m2],
                                     scale=cst_sb[:, 12 + m2:13 + m2])
                nc.vector.tensor_scalar(u4_sb[:, m2, :], u4_sb[:, m2, :],
                                        1.0, -1.0, ALU.min, ALU.max)
                nc.vector.tensor_copy(s4h_sb[:, m2, :], u4_sb[:, m2, :])
                nc.vector.tensor_sub(s4r_sb[:, m2, :], u4_sb[:, m2, :],
                                     s4h_sb[:, m2, :])
                nc.vector.tensor_copy(s4l_sb[:, m2, :], s4r_sb[:, m2, :])

            # fc3 + log_softmax; batch tile bt == g block of fc1 columns.
            h3s, mxs, negs, ses, lss = [], [], [], [], []
            for bt in range(2):
                ph = pp2.tile([128, 512], F32, tag="c2")
                mms = []
                for kt in range(2):
                    lh = s4h_sb[:, kt, 128 * bt:128 * (bt + 1)]
                    ll = s4l_sb[:, kt, 128 * bt:128 * (bt + 1)]
                    w3hv = sbf_sb[:, 128 + 10 * kt:138 + 10 * kt]
                    w3lv = sbf_sb[:, 148 + 10 * kt:158 + 10 * kt]
                    mms += [(lh, w3hv), (ll, w3hv), (lh, w3lv)]
                for i, (lhs, rhs) in enumerate(mms):
                    nc.tensor.matmul(ph[:, :10], lhs, rhs,
                                     start=(i == 0), stop=(i == len(mms) - 1))
                h3 = wk.tile([128, 10], F32, tag="h3", name=f"h3_{bt}")
                nc.vector.tensor_add(h3[:], ph[:, :10], cst_sb[:, 16:26])
                mx = wk.tile([128, 1], F32, tag="mx", name=f"mx_{bt}")
                nc.vector.tensor_reduce(mx[:], h3[:], mybir.AxisListType.X,
                                        ALU.max)
                negmx = wk.tile([128, 1], F32, tag="negmx", name=f"negmx_{bt}")
                nc.vector.tensor_scalar_mul(negmx[:], mx[:], -1.0)
                h3s.append(h3)
                mxs.append(mx)
                negs.append(negmx)
            for bt in range(2):     # both Exps together (one act table set)
                et = wk.tile([128, 10], F32, tag="et", name=f"et_{bt}")
                se = wk.tile([128, 1], F32, tag="se", name=f"se_{bt}")
                nc.scalar.activation(et[:], h3s[bt][:], AF.Exp,
                                     bias=negs[bt][:], scale=1.0,
                                     accum_out=se[:])
                ses.append(se)
            for bt in range(2):     # then both Lns (single table reload)
                ls = wk.tile([128, 1], F32, tag="ls", name=f"ls_{bt}")
                nc.scalar.activation(ls[:], ses[bt][:], AF.Ln)
                lss.append(ls)
            for bt in range(2):
                tt = wk.tile([128, 1], F32, tag="tt", name=f"tt_{bt}")
                nc.vector.tensor_add(tt[:], mxs[bt][:], lss[bt][:])
                o = wk.tile([128, 10], F32, tag="o", name=f"o_{bt}")
                nc.vector.tensor_scalar_sub(o[:], h3s[bt][:], tt[:])
                # un-permute: psum partition p -> sample row
                # s = 8*(p//4) + 4*bt + p%4
                dst = bass.AP(tensor=d_out, offset=bt * 40,
                              ap=[[80, 32], [10, 4], [1, 10]])
                (nc.sync if bt == 0 else nc.scalar).dma_start(dst, o[:])

    nc.compile()
    return nc


def _get_module():
    if "nc" not in _nc_cache:
        nc = _build_module()
        # the module is frozen after compile; memoize its (identical)
        # serialization so per-call re-jits don't re-serialize ~30ms worth
        js = nc.to_json_bytes()
        nc.to_json_bytes = lambda: js
        _nc_cache["nc"] = nc
    return _nc_cache["nc"]


def kernel(**inputs):
    from concourse.bass_utils import run_bass_kernel_spmd

    in_maps = _host_prep(inputs)
    nc = _get_module()
    res = run_bass_kernel_spmd(nc, in_maps, core_ids=list(range(NCORES)))
    out = np.concatenate([r["out"] for r in res.results], axis=0)
    return out.astype(np.float32)


# revision 47
# speedup vs baseline: 1.3752x; 1.0576x over previous
"""BinarizedLeNet5/CIFAR10 Trainium2 kernel (8-core data parallel), v4.

The graded metric is wall-clock of a warm run_bass_kernel_spmd call, and
the axon tunnel moves ~50 MB/s — so v2's 287 MB of host-staged im2col
inputs WAS the runtime.  v4 ships ~16.5 MB instead:

- x goes up as int20 fixed-point over [-6,6): u16 hi + nibble-packed u4
  mantissa extension, unpadded, samples innermost.  The im2col expansion
  happens on-device with 36 strided gather DMAs per chunk-half (3-dim
  APs, contiguous (col,s) runs); halo positions keep a one-time q0
  memset that decodes to exact 0.  DVE unpacks the nibbles, decodes to
  f32 and splits into the bf16 hi/lo pair the conv1 matmuls consume
  (the split residual ~x*2^-18 dominates the encoding error; rel err
  1.12e-2 vs the 2e-2 gate).  conv1 psum free layout is (r, w, s).
- fc1w+fc2w ship bit-packed in one [128,2176] u8 tensor, unpacked
  on-device with DVE shift/and ops.
- all weight tensors arrive 8-way column-sharded and are AllGather'd
  on-device (replicated upload would cost 8x the bytes).
- small f32 constants consolidate into one [128,32] tensor, w1/w3h/w3l
  into one [128,168] bf16 tensor: 6 input args total.

Everything downstream of conv1's ACT-sign (sp/ic2 build, conv2, fc
phase, log_softmax tail) is byte-identical to v2.  A persistent XLA
compilation cache is configured at import so warm calls skip the
~0.5s client-side BIR re-verification that run_bass_kernel_spmd's
per-call re-jit otherwise pays.
"""
import sys
import numpy as np

sys.path.insert(0, "/opt/pypackages")
sys.path.insert(0, "/opt/trn_rl_repo")

import ml_dtypes

# Persistent XLA compilation cache: run_bass_kernel_spmd re-jits on every
# call (fresh closure), and without this each warm call pays ~0.5s of
# client-side BIR re-verification + DVE table gen before the NEFF cache
# hits.  The persistent cache keys on HLO fingerprint, so call 2+ skips
# backend compile entirely.
import jax

jax.config.update("jax_compilation_cache_dir", "/tmp/jax_comp_cache")
jax.config.update("jax_persistent_cache_min_entry_size_bytes", -1)
jax.config.update("jax_persistent_cache_min_compile_time_secs", 0.0)

BF = ml_dtypes.bfloat16
F8 = ml_dtypes.float8_e4m3
NCORES = 8
B = 2048
BC = B // NCORES          # 256 samples per core
CH = 16                   # samples per chunk
NCHUNK = BC // CH         # 16 chunks
EPS = np.float32(1e-5)
XSPLIT = 1                # x upload streams (1 measured fastest)
XCHW = 3 * 32 * 32 * CH   # chunk elements, unpadded (49152)
XCHM = XCHW // 2          # nibble-packed chunk bytes (24576)
XSC = float(2 ** 20) / 12.0   # int20 fixed-point scale over [-6, 6)
Q0 = 1 << 19                  # encoding of x == 0.0
C_HI = 3.0 * 2.0 ** -14       # decode: x = hi16*C_HI + m4*C_LO - 6
C_LO = 3.0 * 2.0 ** -18

_nc_cache = {}


def _f32(x):
    return np.asarray(x, np.float32)


def _host_prep(inputs):
    """Build all per-core device input arrays (layout prep only)."""
    x = _f32(inputs["x"])                      # [2048,3,32,32]

    # ---- x as int20 fixed-point over [-6,6): u16 hi + u4 nibble ext,
    # unpadded, samples innermost (halo positions stay at the q0 memset) ----
    q = np.clip(np.round((x.astype(np.float64) + 6.0) * XSC),
                0, 2 ** 20 - 1).astype(np.uint32)

    def stage(xq):
        xt = xq.reshape(NCORES, NCHUNK, CH, 3, 32, 32)
        xt = np.ascontiguousarray(xt.transpose(0, 1, 3, 4, 5, 2))
        return xt.reshape(NCORES, NCHUNK, -1)

    xh = stage((q >> 4).astype(np.uint16))
    m4 = stage((q & 15).astype(np.uint8))       # [.., pos, s]
    m4 = m4.reshape(NCORES, NCHUNK, XCHM, 2)    # byte = (s even, s odd)
    xl = (m4[..., 0] | (m4[..., 1] << 4)).astype(np.uint8)
    # one u16 arg per core: [hi16 | nibble bytes viewed as u16 pairs]
    xq = np.concatenate(
        [xh, np.ascontiguousarray(xl).view(np.uint16)], axis=2)

    # ---- conv1 stationary: block-diag, k = 3*(3dy+dx)+c, out p = 4*co+j ----
    w1s = np.sign(_f32(inputs["conv1_w"]))               # [32,3,3,3]
    w1k = np.ascontiguousarray(w1s.transpose(2, 3, 1, 0)).reshape(27, 32)
    # bf16 blob: [:,0:128]=w1, [:,128:148]=w3h (kt-major), [:,148:168]=w3l
    wbf = np.zeros((128, 168), BF)
    for j in range(4):
        for co in range(32):
            wbf[32 * j:32 * j + 27, 4 * co + j] = w1k[:, co].astype(BF)
    w3 = _f32(inputs["fc3_w"]).T                         # [256,10]
    w3h = w3.astype(BF)
    w3l = (w3 - w3h.astype(np.float32)).astype(BF)
    for kt in range(2):
        wbf[:, 128 + 10 * kt:138 + 10 * kt] = w3h[128 * kt:128 * (kt + 1)]
        wbf[:, 148 + 10 * kt:158 + 10 * kt] = w3l[128 * kt:128 * (kt + 1)]

    # ---- conv2 stationaries [3][96,64]: p = 32*dx + c (bit-packed) ----
    w2s = np.sign(_f32(inputs["conv2_w"]))               # [64,32,3,3]
    w2_st = np.zeros((3, 96, 64), np.float32)
    for dy in range(3):
        for dx in range(3):
            w2_st[dy, 32 * dx:32 * dx + 32] = w2s[:, :, dy, dx].T

    # ---- consolidated f32 constants [128, 32] (padded for 8-way shard) ----
    cst = np.zeros((128, 32), np.float32)
    inv1 = _f32(inputs["bn1_g"]) / np.sqrt(_f32(inputs["bn1_v"]) + EPS)
    sh1c = (_f32(inputs["conv1_b"]) - _f32(inputs["bn1_m"])) * inv1 \
        + _f32(inputs["bn1_b"])
    cst[:, 0] = np.repeat(inv1, 4)
    cst[:, 1] = np.repeat(sh1c, 4)
    inv2 = _f32(inputs["bn2_g"]) / np.sqrt(_f32(inputs["bn2_v"]) + EPS)
    sh2c = _f32(inputs["bn2_b"]) - _f32(inputs["bn2_m"]) * inv2
    cst[:, 2] = np.tile(inv2, 2)
    cst[:, 3] = np.tile(_f32(inputs["conv2_b"]) * inv2 + sh2c, 2)
    inv3 = _f32(inputs["bn3_g"]) / np.sqrt(_f32(inputs["bn3_v"]) + EPS)
    sh3c = (_f32(inputs["fc1_b"]) - _f32(inputs["bn3_m"])) * inv3 \
        + _f32(inputs["bn3_b"])
    cst[:, 4:8] = inv3.reshape(4, 128).T
    cst[:, 8:12] = sh3c.reshape(4, 128).T
    inv4 = _f32(inputs["bn4_g"]) / np.sqrt(_f32(inputs["bn4_v"]) + EPS)
    sh4c = (_f32(inputs["fc2_b"]) - _f32(inputs["bn4_m"])) * inv4 \
        + _f32(inputs["bn4_b"])
    cst[:, 12:14] = inv4.reshape(2, 128).T
    cst[:, 14:16] = sh4c.reshape(2, 128).T
    cst[:, 16:26] = _f32(inputs["fc3_b"])[None, :]

    # ---- fc1+fc2+w2 bit-packed into one [128, 2200] u8 tensor ----
    # fc1: kt = yx//2, kp = c + 64*(yx%2), bits along m (cols 0:2048);
    # fc2 at 2048:2176; w2 (rows 0:96 only) at 2176:2200
    fw1 = np.sign(_f32(inputs["fc1_w"]))                 # [512,4096]
    A = fw1.T.reshape(64, 64, 512)                       # [c][yx][m]
    Bm = A.reshape(64, 32, 2, 512)                       # [c][kt][yxp][m]
    fc1_st = np.ascontiguousarray(
        Bm.transpose(1, 2, 0, 3).reshape(32, 128, 512))  # [kt][kp][m]
    fw2 = np.sign(_f32(inputs["fc2_w"]))                 # [256,512]
    fc2_st = np.ascontiguousarray(fw2.T.reshape(4, 128, 256))
    w2v = np.zeros((128, 192), np.float32)               # [p][dy*64+m]
    w2v[:96] = np.ascontiguousarray(
        w2_st.transpose(1, 0, 2)).reshape(96, 192)

    wpk = np.zeros((128, 2200), np.uint8)
    for st, off in ((fc1_st.transpose(1, 0, 2).reshape(128, -1), 0),
                    (fc2_st.transpose(1, 0, 2).reshape(128, -1), 2048),
                    (w2v, 2176)):
        bits = (st > 0).astype(np.uint8)
        for b in range(8):
            wpk[:, off:off + st.shape[1] // 8] |= (bits[:, b::8]
                                                   << np.uint8(b))

    # cst ships as a bf16 hi/lo pair alongside wbf (one bf16 arg)
    ch = cst.astype(BF)
    cl = (cst - ch.astype(np.float32)).astype(BF)
    sbf = np.concatenate([wbf, ch, cl], axis=1)          # [128, 232]

    # weights ship 8-way column-sharded and AllGather on device; x ships
    # split into XSPLIT args (a few parallel tunnel streams beat one)
    PER = NCHUNK // XSPLIT
    in_maps = []
    for ci in range(NCORES):
        m = {
            "s8": np.ascontiguousarray(wpk[:, 275 * ci:275 * (ci + 1)]),
            "sbf": np.ascontiguousarray(sbf[:, 29 * ci:29 * (ci + 1)]),
        }
        for k in range(XSPLIT):
            m[f"xq{k}"] = np.ascontiguousarray(xq[ci, k * PER:(k + 1) * PER])
        in_maps.append(m)
    return in_maps


def _build_module(reps=1):
    import concourse.bass as bass
    import concourse.mybir as mybir
    import concourse.tile as tile
    from concourse import bacc
    from contextlib import ExitStack

    F32 = mybir.dt.float32
    BF16 = mybir.dt.bfloat16
    FP8 = mybir.dt.float8e4
    U8 = mybir.dt.uint8
    AF = mybir.ActivationFunctionType
    ALU = mybir.AluOpType

    nc = bacc.Bacc("TRN2", target_bir_lowering=False, debug=False,
                   num_devices=NCORES)

    U16 = mybir.dt.uint16
    RG = [list(range(NCORES))]

    # ---- DRAM tensors: weights arrive 8-way sharded, AllGather on-device;
    # x is one u16 tensor per core: [hi16 | nibble-pairs] per chunk row ----
    XROW = XCHW + XCHM // 2
    d_xq = [nc.dram_tensor(f"xq{k}", [NCHUNK // XSPLIT, XROW], U16,
                           kind="ExternalInput") for k in range(XSPLIT)]
    shards = {}
    for nm, shp, dt in (("s8", [128, 275], U8), ("sbf", [128, 29], BF16)):
        d_s = nc.dram_tensor(nm, shp, dt, kind="ExternalInput")
        d_i = nc.dram_tensor("i" + nm, shp, dt, kind="Internal")
        d_g = nc.dram_tensor("g" + nm, [NCORES] + shp, dt, kind="Internal",
                             addr_space="Shared")
        shards[nm] = (d_s, d_i, d_g)
    d_out = nc.dram_tensor("out", [BC, 10], F32, kind="ExternalOutput")

    NBUF = 3                       # sp/ic2 ping-pong depth
    NXB = 2                        # ic1 staging ping-pong depth
    SPW = 4 * CH * 18              # sp payload per partition
    SP_FREE = SPW + 8
    IC2_FREE = 18 * CH * 18 + 8    # (R 18, s CH, W 18)

    with tile.TileContext(nc) as tc, ExitStack() as ctx:
        const = ctx.enter_context(tc.tile_pool(name="const", bufs=1))
        wk = ctx.enter_context(tc.tile_pool(name="wk", bufs=3))
        pp1 = ctx.enter_context(tc.tile_pool(name="pp1", bufs=3, space="PSUM"))
        pp2 = ctx.enter_context(tc.tile_pool(name="pp2", bufs=2, space="PSUM"))

        # ---- persistent tiles ----
        sbf_sb = const.tile([128, 232], BF16, tag="sbf")
        w2_sb = const.tile([96, 3, 64], FP8, tag="w2")
        fc1w_sb = const.tile([128, 32, 512], FP8, tag="fc1w")
        fc2w_sb = const.tile([128, 4, 256], FP8, tag="fc2w")
        cst_sb = const.tile([128, 32], F32, tag="cst")
        pk_sb = const.tile([128, 2200], U8, tag="pk")
        tmp_sb = const.tile([128, 2200], U8, tag="tmp")
        qh_t = [const.tile([128, 4096], U16, tag=f"qh{i}", name=f"qh{i}")
                for i in range(NXB)]
        qm_t = [const.tile([128, 1024], U16, tag=f"qm{i}", name=f"qm{i}")
                for i in range(NXB)]
        qmu_sb = const.tile([128, 4096], U16, tag="qmu")
        ich_t = [const.tile([128, 4096], BF16, tag=f"ich{i}", name=f"ich{i}")
                 for i in range(NXB)]
        icl_t = [const.tile([128, 4096], BF16, tag=f"icl{i}", name=f"icl{i}")
                 for i in range(NXB)]
        tf_sb = const.tile([128, 4096], F32, tag="tf")
        tm_sb = const.tile([128, 4096], F32, tag="tm")
        sp_t = [const.tile([128, SP_FREE], FP8, tag=f"sp{i}", name=f"sp{i}")
                for i in range(NBUF)]
        ic2_t = [const.tile([96, IC2_FREE], FP8, tag=f"ic2_{i}", name=f"ic2_{i}")
                 for i in range(NBUF)]
        # s2all: partition (g2, c64), free = yx*128 + sh,
        #   sh = (2*chk + tau)*4 + sl,  sample s = 8*(sh//4) + 4g + sh%4
        s2all = const.tile([128, 64 * 128], FP8, tag="s2all")
        # fc1in: partition kp = c + 64*(yx%2), free = kt*BC + (g*128 + sh)
        fc1in = const.tile([128, 32 * BC], FP8, tag="fc1in")
        s3_sb = const.tile([128, 4, BC], FP8, tag="s3")
        u4_sb = const.tile([128, 2, BC], F32, tag="u4")
        s4h_sb = const.tile([128, 2, BC], BF16, tag="s4h")
        s4l_sb = const.tile([128, 2, BC], BF16, tag="s4l")
        s4r_sb = const.tile([128, 2, BC], F32, tag="s4r")

        w1v = sbf_sb[:, 0:128]

        def ap_of(t, dims, off=0):
            return bass.AP(tensor=t.tensor, offset=t.offset + off,
                           ap=[list(t.ap[0])] + [list(d) for d in dims])

        # ---- setup: AllGather the weight shards, then load to SBUF ----
        for nm, (d_s, d_i, d_g) in shards.items():
            nc.gpsimd.dma_start(d_i.ap(), d_s.ap())
            nc.gpsimd.collective_compute(
                "AllGather", ALU.bypass, RG, ins=[d_i.ap()], outs=[d_g.ap()])
        for c in range(NCORES):
            g = shards["s8"][2].ap()[c]
            nc.gpsimd.dma_start(pk_sb[:, 275 * c:275 * (c + 1)], g)
            g = shards["sbf"][2].ap()[c]
            nc.gpsimd.dma_start(sbf_sb[:, 29 * c:29 * (c + 1)], g)
        # cst = bf16 hi + bf16 lo
        nc.vector.tensor_add(cst_sb[:], sbf_sb[:, 168:200],
                             sbf_sb[:, 200:232])

        # fc1w/fc2w/w2 unpack: bit b of packed byte mb -> m = 8*mb + b
        for b in range(8):
            nc.vector.tensor_scalar(tmp_sb[:], pk_sb[:], b, 1,
                                    ALU.logical_shift_right, ALU.bitwise_and)
            dst = bass.AP(tensor=fc1w_sb.tensor, offset=fc1w_sb.offset + b,
                          ap=[list(fc1w_sb.ap[0]), [512, 32], [8, 64]])
            nc.vector.tensor_scalar(dst, bass.AP(
                tensor=tmp_sb.tensor, offset=tmp_sb.offset,
                ap=[list(tmp_sb.ap[0]), [1, 2048]]), 2.0, -1.0,
                ALU.mult, ALU.add)
            dst2 = bass.AP(tensor=fc2w_sb.tensor, offset=fc2w_sb.offset + b,
                           ap=[list(fc2w_sb.ap[0]), [256, 4], [8, 32]])
            nc.vector.tensor_scalar(dst2, bass.AP(
                tensor=tmp_sb.tensor, offset=tmp_sb.offset + 2048,
                ap=[list(tmp_sb.ap[0]), [1, 128]]), 2.0, -1.0,
                ALU.mult, ALU.add)
            dst3 = bass.AP(tensor=w2_sb.tensor, offset=w2_sb.offset + b,
                           ap=[list(w2_sb.ap[0]), [8, 24]])
            tv = tmp_sb[0:96]
            nc.vector.tensor_scalar(dst3, bass.AP(
                tensor=tv.tensor, offset=tv.offset + 2176,
                ap=[list(tv.ap[0]), [1, 24]]), 2.0, -1.0,
                ALU.mult, ALU.add)

        # init the gather tiles to the encoding of 0.0 once: pad partitions
        # and clipped row slabs stay at q0 forever and decode to exact 0.
        for t in qh_t:
            nc.vector.memset(t[:], Q0 >> 4)
        for t in qm_t:
            nc.vector.memset(t[:], 0)
        # sp pads: cols w==0 and w==17 of each 18-block, plus the 8 slack cols
        for t in sp_t:
            nc.vector.memset(
                ap_of(t, [[18, SPW // 18], [17, 2], [1, 1]]), 0.0)
            nc.vector.memset(ap_of(t, [[1, 8]], SPW), 0.0)
        # ic2 pads: halo rows R=0 and R=17 (+ slack)
        for t in ic2_t:
            nc.vector.memset(
                ap_of(t, [[17 * CH * 18, 2], [1, CH * 18]]), 0.0)
            nc.vector.memset(ap_of(t, [[1, 8]], 18 * CH * 18), 0.0)

        dma_engs = [nc.sync, nc.scalar, nc.gpsimd]

        for _rep in range(reps):
            # ================= chunk loop (software-pipelined) =================
            # iteration k emits conv1 of chunk k, then conv2 of chunk k-1, so
            # the in-order PE queue never stalls on chunk k-1's ic2 DMAs.
            def conv1_part(chk):
                sp = sp_t[chk % NBUF]
                ic2 = ic2_t[chk % NBUF]
                qh = qh_t[chk % NXB]
                qm = qm_t[chk % NXB]
                ich = ich_t[chk % NXB]
                icl = icl_t[chk % NXB]

                # ---- on-device im2col: 36 gather DMAs per half ----
                # dst w-range / src col-range clip at the image edge; the
                # unwritten halo positions keep their q0 memset (decode 0).
                qi = 0
                for tgt, base, G in ((qh, 0, 16), (qm, XCHW, 4)):
                    for j in range(4):
                        for dy in range(3):
                            r0, nr = 0, 8
                            if j == 0 and dy == 0:
                                r0, nr = 1, 7
                            if j == 3 and dy == 2:
                                r0, nr = 0, 7
                            PER = NCHUNK // XSPLIT
                            dxt = d_xq[chk // PER]
                            soff = (chk % PER) * XROW + base \
                                + (8 * j + dy + r0 - 1) * (32 * G)
                            for dx in range(3):
                                p0 = 32 * j + 9 * dy + 3 * dx
                                run = 32 * G if dx == 1 else 31 * G
                                pv = tgt[p0:p0 + 3]
                                dst = bass.AP(
                                    tensor=pv.tensor,
                                    offset=pv.offset + r0 * 32 * G
                                    + (G if dx == 0 else 0),
                                    ap=[list(pv.ap[0]), [32 * G, nr],
                                        [1, run]])
                                src = bass.AP(
                                    tensor=dxt,
                                    offset=soff + (G if dx == 2 else 0),
                                    ap=[[1024 * G, 3], [32 * G, nr],
                                        [1, run]])
                                dma_engs[qi % 3].dma_start(dst, src)
                                qi += 1

                # ---- nibble unpack + int20 decode + bf16 hi/lo split ----
                for n in range(4):
                    nc.vector.tensor_scalar(
                        ap_of(qmu_sb, [[16, 256], [4, 4]], n), qm[:],
                        4 * n, 15, ALU.logical_shift_right, ALU.bitwise_and)
                nc.vector.tensor_scalar(tf_sb[:], qh[:], C_HI, None, ALU.mult)
                nc.vector.tensor_scalar(tm_sb[:], qmu_sb[:], C_LO, -6.0,
                                        ALU.mult, ALU.add)
                nc.vector.tensor_add(tf_sb[:], tf_sb[:], tm_sb[:])
                nc.vector.tensor_copy(ich[:], tf_sb[:])
                nc.vector.tensor_sub(tm_sb[:], tf_sb[:], ich[:])
                nc.vector.tensor_copy(icl[:], tm_sb[:])

                # ---- conv1: 4 psum tiles; tile tau = pooled row pair ----
                # psum free = dr*512 + w*16 + s
                for tau in range(4):
                    p1 = pp1.tile([128, 1024], F32, tag="c1")
                    for dr in range(2):
                        sl = bass.ds((2 * tau + dr) * 512, 512)
                        nc.tensor.matmul(p1[:, dr * 512:(dr + 1) * 512],
                                         w1v, ich[:, sl],
                                         start=True, stop=False)
                        nc.tensor.matmul(p1[:, dr * 512:(dr + 1) * 512],
                                         w1v, icl[:, sl],
                                         start=False, stop=True)
                    # DVE: 2x2 maxpool in ONE XY-window reduce from psum
                    # out free = 16*s + wp
                    pl1 = wk.tile([128, 256], F32, tag="pl1")
                    nc.vector.tensor_reduce(
                        ap_of(pl1, [[16, 16], [1, 16]]),
                        ap_of(p1, [[1, 16], [32, 16], [512, 2], [16, 2]]),
                        mybir.AxisListType.XY, ALU.max)
                    # ACT: sign(bn1) -> +-1 fp8 straight into sp
                    # sp free = rr*288 + s*18 + (wp+1), rr = tau
                    nc.scalar.activation(
                        ap_of(sp, [[18, 16], [1, 16]], tau * 288 + 1),
                        pl1[:], AF.Sign,
                        bias=cst_sb[:, 1:2], scale=cst_sb[:, 0:1])

                # ---- ic2 build: 3 contiguous-run DMAs (SP, ACT, Pool) ----
                RUN = 4 * CH * 18
                for dx in range(3):
                    src = bass.AP(tensor=sp.tensor, offset=sp.offset + dx,
                                  ap=[list(sp.ap[0]), [1, RUN]])
                    dst_t = ic2[32 * dx:32 * (dx + 1)]
                    dst = bass.AP(tensor=dst_t.tensor,
                                  offset=dst_t.offset + CH * 18,
                                  ap=[list(dst_t.ap[0]), [RUN, 4], [1, RUN]])
                    eng = (nc.sync, nc.scalar, nc.gpsimd)[dx]
                    eng.dma_start(dst, src)

            def conv2_part(chk):
                ic2 = ic2_t[chk % NBUF]
                # ---- conv2: 4 one-bank col-packed psum tiles (tau, h) ----
                for tau in range(2):
                    for h in range(2):
                        p2 = pp2.tile([128, 512], F32, tag="c2")
                        for g in range(2):
                            tp = (0, 64 * g) if g else None
                            for dy in range(3):
                                s0 = 8 * tau + 4 * g + 2 * h
                                mv = bass.AP(
                                    tensor=ic2.tensor,
                                    offset=ic2.offset + s0 * 18
                                    + dy * (CH * 18),
                                    ap=[list(ic2.ap[0]), [18, 2],
                                        [CH * 18, 16], [1, 16]])
                                nc.tensor.matmul(
                                    p2[64 * g:64 * (g + 1), :],
                                    w2_sb[:, dy, :], mv,
                                    start=(dy == 0), stop=(dy == 2),
                                    tile_position=tp)
                        # DVE: 2x2 maxpool, one XY-window reduce
                        xm2b = wk.tile([128, 128], F32, tag="xm2b")
                        nc.vector.tensor_reduce(
                            ap_of(xm2b, [[8, 16], [1, 8]]),
                            ap_of(p2, [[32, 16], [2, 8], [16, 2], [1, 2]]),
                            mybir.AxisListType.XY, ALU.max)
                        # ACT sign(bn2) -> +-1 fp8 into s2all
                        sh0 = (2 * chk + tau) * 4 + 2 * h
                        nc.scalar.activation(
                            ap_of(s2all, [[1, 2], [1024, 8], [128, 8]], sh0),
                            xm2b[:], AF.Sign,
                            bias=cst_sb[:, 3:4], scale=cst_sb[:, 2:3])

                # ---- repack waves: one DMA per (g, yxp, wave) ----
                # waves at chk 7 (sh 0:64), 11 (64:96), 15 (96:128) so the
                # final fc1 dependency is only a quarter-size transfer
                WAVES = {7: (0, 64), 11: (64, 32), 15: (96, 32)}
                if chk in WAVES:
                    lo, ln = WAVES[chk]
                    for g in range(2):
                        for yxp in range(2):
                            src_t = s2all[64 * g:64 * (g + 1)]
                            src = bass.AP(
                                tensor=src_t.tensor,
                                offset=src_t.offset + 128 * yxp + lo,
                                ap=[list(src_t.ap[0]), [256, 32], [1, ln]])
                            dst_t = fc1in[64 * yxp:64 * (yxp + 1)]
                            dst = bass.AP(
                                tensor=dst_t.tensor,
                                offset=dst_t.offset + g * 128 + lo,
                                ap=[list(dst_t.ap[0]), [256, 32], [1, ln]])
                            eng = (nc.sync, nc.scalar, nc.gpsimd,
                                   nc.gpsimd)[2 * g + yxp]
                            eng.dma_start(dst, src)

            conv1_part(0)
            for chk in range(1, NCHUNK):
                conv1_part(chk)
                conv2_part(chk - 1)
            conv2_part(NCHUNK - 1)

            # ================= fc phase =================
            # fc1: 4 m-tiles, fp8 DoubleRow over kt pairs, column-sliced:
            # cols filled by waves 1+2 (sh 0:96 of both g-blocks) run while
            # wave 3 is still transferring; the wave-3 cols follow.
            pfs = []
            for m in range(4):
                pf = (pp1 if m < 3 else pp2).tile(
                    [128, 1024] if m < 3 else [128, 512], F32,
                    tag="c1" if m < 3 else "c2", name=f"pf_{m}")
                pfs.append(pf)
            SLICES = [[(0, 96), (128, 96)], [(96, 32), (224, 32)]]
            for phase in range(2):
                for m in range(4):
                    pf = pfs[m]
                    for c0, wdt in SLICES[phase]:
                        for kt in range(16):
                            lw = bass.AP(
                                tensor=fc1w_sb.tensor,
                                offset=fc1w_sb.offset + 2 * kt * 512
                                + 128 * m,
                                ap=[list(fc1w_sb.ap[0]), [512, 2], [1, 128]])
                            mv = bass.AP(
                                tensor=fc1in.tensor,
                                offset=fc1in.offset + 2 * kt * BC + c0,
                                ap=[list(fc1in.ap[0]), [BC, 2], [1, wdt]])
                            nc.tensor.matmul(
                                pf[:, c0:c0 + wdt], lw, mv,
                                start=(kt == 0), stop=(kt == 15),
                                perf_mode=mybir.MatmulPerfMode.DoubleRow)
            for m in range(4):
                nc.scalar.activation(s3_sb[:, m, :], pfs[m][:, :BC], AF.Sign,
                                     bias=cst_sb[:, 8 + m:9 + m],
                                     scale=cst_sb[:, 4 + m:5 + m])

            # fc2: 2 m-tiles, fp8 DoubleRow over kt pairs; clip + hi/lo
            # split per m2-half so the DVE chain overlaps fc2's second half
            for m2 in range(2):
                pg = pp1.tile([128, 1024], F32, tag="c1")
                for kt in range(2):
                    lw = bass.AP(
                        tensor=fc2w_sb.tensor,
                        offset=fc2w_sb.offset + 2 * kt * 256 + 128 * m2,
                        ap=[list(fc2w_sb.ap[0]), [256, 2], [1, 128]])
                    mv = bass.AP(
                        tensor=s3_sb.tensor,
                        offset=s3_sb.offset + 2 * kt * BC,
                        ap=[list(s3_sb.ap[0]), [BC, 2], [1, BC]])
                    nc.tensor.matmul(pg[:, :BC], lw, mv,
                                     start=(kt == 0), stop=(kt == 1),
                                     perf_mode=mybir.MatmulPerfMode.DoubleRow)
                nc.scalar.activation(u4_sb[:, m2, :], pg[:, :BC], AF.Identity,
                                     bias=cst_sb[:, 14 + m2:15 + m2],
                                     scale=cst_sb[:, 12 + m2:13 + m2])
                nc.vector.tensor_scalar(u4_sb[:, m2, :], u4_sb[:, m2, :],
                                        1.0, -1.0, ALU.min, ALU.max)
                nc.vector.tensor_copy(s4h_sb[:, m2, :], u4_sb[:, m2, :])
                nc.vector.tensor_sub(s4r_sb[:, m2, :], u4_sb[:, m2, :],
                                     s4h_sb[:, m2, :])
                nc.vector.tensor_copy(s4l_sb[:, m2, :], s4r_sb[:, m2, :])

            # fc3 + log_softmax; batch tile bt == g block of fc1 columns.
            h3s, mxs, negs, ses, lss = [], [], [], [], []
            for bt in range(2):
                ph = pp2.tile([128, 512], F32, tag="c2")
                mms = []
                for kt in range(2):
                    lh = s4h_sb[:, kt, 128 * bt:128 * (bt + 1)]
                    ll = s4l_sb[:, kt, 128 * bt:128 * (bt + 1)]
                    w3hv = sbf_sb[:, 128 + 10 * kt:138 + 10 * kt]
                    w3lv = sbf_sb[:, 148 + 10 * kt:158 + 10 * kt]
                    mms += [(lh, w3hv), (ll, w3hv), (lh, w3lv)]
                for i, (lhs, rhs) in enumerate(mms):
                    nc.tensor.matmul(ph[:, :10], lhs, rhs,
                                     start=(i == 0), stop=(i == len(mms) - 1))
                h3 = wk.tile([128, 10], F32, tag="h3", name=f"h3_{bt}")
                nc.vector.tensor_add(h3[:], ph[:, :10], cst_sb[:, 16:26])
                mx = wk.tile([128, 1], F32, tag="mx", name=f"mx_{bt}")
                nc.vector.tensor_reduce(mx[:], h3[:], mybir.AxisListType.X,
                                        ALU.max)
                negmx = wk.tile([128, 1], F32, tag="negmx", name=f"negmx_{bt}")
                nc.vector.tensor_scalar_mul(negmx[:], mx[:], -1.0)
                h3s.append(h3)
                mxs.append(mx)
                negs.append(negmx)
            for bt in range(2):     # both Exps together (one act table set)
                et = wk.tile([128, 10], F32, tag="et", name=f"et_{bt}")
                se = wk.tile([128, 1], F32, tag="se", name=f"se_{bt}")
                nc.scalar.activation(et[:], h3s[bt][:], AF.Exp,
                                     bias=negs[bt][:], scale=1.0,
                                     accum_out=se[:])
                ses.append(se)
            for bt in range(2):     # then both Lns (single table reload)
                ls = wk.tile([128, 1], F32, tag="ls", name=f"ls_{bt}")
                nc.scalar.activation(ls[:], ses[bt][:], AF.Ln)
                lss.append(ls)
            for bt in range(2):
                tt = wk.tile([128, 1], F32, tag="tt", name=f"tt_{bt}")
                nc.vector.tensor_add(tt[:], mxs[bt][:], lss[bt][:])
                o = wk.tile([128, 10], F32, tag="o", name=f"o_{bt}")
                nc.vector.tensor_scalar_sub(o[:], h3s[bt][:], tt[:])
                # un-permute: psum partition p -> sample row
                # s = 8*(p//4) + 4*bt + p%4
                dst = bass.AP(tensor=d_out, offset=bt * 40,
                              ap=[[80, 32], [10, 4], [1, 10]])
                (nc.sync if bt == 0 else nc.scalar).dma_start(dst, o[:])

    nc.compile()
    return nc


def _get_module():
    if "nc" not in _nc_cache:
        nc = _build_module()
        # the module is frozen after compile; memoize its (identical)
        # serialization so per-call re-jits don't re-serialize ~30ms worth
        js = nc.to_json_bytes()
        nc.to_json_bytes = lambda: js
        _nc_cache["nc"] = nc
    return _nc_cache["nc"]


def kernel(**inputs):
    from concourse.bass_utils import run_bass_kernel_spmd

    in_maps = _host_prep(inputs)
    nc = _get_module()
    res = run_bass_kernel_spmd(nc, in_maps, core_ids=list(range(NCORES)))
    out = np.concatenate([r["out"] for r in res.results], axis=0)
    return out.astype(np.float32)


# revision 48
# speedup vs baseline: 1.4218x; 1.0340x over previous
"""BinarizedLeNet5/CIFAR10 Trainium2 kernel (8-core data parallel), v4.

The graded metric is wall-clock of a warm run_bass_kernel_spmd call, and
the axon tunnel moves ~50 MB/s — so v2's 287 MB of host-staged im2col
inputs WAS the runtime.  v4 ships ~16.5 MB instead:

- x goes up as int20 fixed-point over [-6,6): u16 hi + nibble-packed u4
  mantissa extension, unpadded, samples innermost.  The im2col expansion
  happens on-device with 36 strided gather DMAs per chunk-half (3-dim
  APs, contiguous (col,s) runs); halo positions keep a one-time q0
  memset that decodes to exact 0.  DVE unpacks the nibbles, decodes to
  f32 and splits into the bf16 hi/lo pair the conv1 matmuls consume
  (the split residual ~x*2^-18 dominates the encoding error; rel err
  1.12e-2 vs the 2e-2 gate).  conv1 psum free layout is (r, w, s).
- fc1w+fc2w+w2 ship bit-packed in one [128,2200] u8 tensor, unpacked
  on-device with DVE shift/and ops.
- all weight tensors arrive 8-way column-sharded and are AllGather'd
  on-device (replicated upload would cost 8x the bytes); the f32
  constants ride as a bf16 hi/lo pair in the bf16 tensor.
- 3 input args total (x, u8 weight bits, bf16 weights) — per-arg
  transfer overhead through the tunnel is ~10ms, so fewer args win.

Everything downstream of conv1's ACT-sign (sp/ic2 build, conv2, fc
phase, log_softmax tail) is byte-identical to v2.  A persistent XLA
compilation cache is configured at import so warm calls skip the
~0.5s client-side BIR re-verification that run_bass_kernel_spmd's
per-call re-jit otherwise pays.
"""
import sys
import numpy as np

sys.path.insert(0, "/opt/pypackages")
sys.path.insert(0, "/opt/trn_rl_repo")

import ml_dtypes

# Persistent XLA compilation cache: run_bass_kernel_spmd re-jits on every
# call (fresh closure), and without this each warm call pays ~0.5s of
# client-side BIR re-verification + DVE table gen before the NEFF cache
# hits.  The persistent cache keys on HLO fingerprint, so call 2+ skips
# backend compile entirely.
import jax

jax.config.update("jax_compilation_cache_dir", "/tmp/jax_comp_cache")
jax.config.update("jax_persistent_cache_min_entry_size_bytes", -1)
jax.config.update("jax_persistent_cache_min_compile_time_secs", 0.0)

BF = ml_dtypes.bfloat16
F8 = ml_dtypes.float8_e4m3
NCORES = 8
B = 2048
BC = B // NCORES          # 256 samples per core
CH = 16                   # samples per chunk
NCHUNK = BC // CH         # 16 chunks
EPS = np.float32(1e-5)
XSPLIT = 1                # x upload streams (1 measured fastest)
XCHW = 3 * 32 * 32 * CH   # chunk elements, unpadded (49152)
XCHM = XCHW // 2          # nibble-packed chunk bytes (24576)
XSC = float(2 ** 20) / 12.0   # int20 fixed-point scale over [-6, 6)
Q0 = 1 << 19                  # encoding of x == 0.0
C_HI = 3.0 * 2.0 ** -14       # decode: x = hi16*C_HI + m4*C_LO - 6
C_LO = 3.0 * 2.0 ** -18

_nc_cache = {}


def _f32(x):
    return np.asarray(x, np.float32)


def _host_prep(inputs):
    """Build all per-core device input arrays (layout prep only)."""
    x = _f32(inputs["x"])                      # [2048,3,32,32]

    # ---- x as int20 fixed-point over [-6,6): u16 hi + u4 nibble ext,
    # unpadded, samples innermost (halo positions stay at the q0 memset) ----
    q = np.clip(np.round((x.astype(np.float64) + 6.0) * XSC),
                0, 2 ** 20 - 1).astype(np.uint32)

    def stage(xq):
        xt = xq.reshape(NCORES, NCHUNK, CH, 3, 32, 32)
        xt = np.ascontiguousarray(xt.transpose(0, 1, 3, 4, 5, 2))
        return xt.reshape(NCORES, NCHUNK, -1)

    xh = stage((q >> 4).astype(np.uint16))
    m4 = stage((q & 15).astype(np.uint8))       # [.., pos, s]
    m4 = m4.reshape(NCORES, NCHUNK, XCHM, 2)    # byte = (s even, s odd)
    xl = (m4[..., 0] | (m4[..., 1] << 4)).astype(np.uint8)
    # one u16 arg per core: [hi16 | nibble bytes viewed as u16 pairs]
    xq = np.concatenate(
        [xh, np.ascontiguousarray(xl).view(np.uint16)], axis=2)

    # ---- conv1 stationary: block-diag, k = 3*(3dy+dx)+c, out p = 4*co+j ----
    w1s = np.sign(_f32(inputs["conv1_w"]))               # [32,3,3,3]
    w1k = np.ascontiguousarray(w1s.transpose(2, 3, 1, 0)).reshape(27, 32)
    # bf16 blob: [:,0:128]=w1, [:,128:148]=w3h (kt-major), [:,148:168]=w3l
    wbf = np.zeros((128, 168), BF)
    for j in range(4):
        for co in range(32):
            wbf[32 * j:32 * j + 27, 4 * co + j] = w1k[:, co].astype(BF)
    w3 = _f32(inputs["fc3_w"]).T                         # [256,10]
    w3h = w3.astype(BF)
    w3l = (w3 - w3h.astype(np.float32)).astype(BF)
    for kt in range(2):
        wbf[:, 128 + 10 * kt:138 + 10 * kt] = w3h[128 * kt:128 * (kt + 1)]
        wbf[:, 148 + 10 * kt:158 + 10 * kt] = w3l[128 * kt:128 * (kt + 1)]

    # ---- conv2 stationaries [3][96,64]: p = 32*dx + c (bit-packed) ----
    w2s = np.sign(_f32(inputs["conv2_w"]))               # [64,32,3,3]
    w2_st = np.zeros((3, 96, 64), np.float32)
    for dy in range(3):
        for dx in range(3):
            w2_st[dy, 32 * dx:32 * dx + 32] = w2s[:, :, dy, dx].T

    # ---- consolidated f32 constants [128, 32] (padded for 8-way shard) ----
    cst = np.zeros((128, 32), np.float32)
    inv1 = _f32(inputs["bn1_g"]) / np.sqrt(_f32(inputs["bn1_v"]) + EPS)
    sh1c = (_f32(inputs["conv1_b"]) - _f32(inputs["bn1_m"])) * inv1 \
        + _f32(inputs["bn1_b"])
    cst[:, 0] = np.repeat(inv1, 4)
    cst[:, 1] = np.repeat(sh1c, 4)
    inv2 = _f32(inputs["bn2_g"]) / np.sqrt(_f32(inputs["bn2_v"]) + EPS)
    sh2c = _f32(inputs["bn2_b"]) - _f32(inputs["bn2_m"]) * inv2
    cst[:, 2] = np.tile(inv2, 2)
    cst[:, 3] = np.tile(_f32(inputs["conv2_b"]) * inv2 + sh2c, 2)
    inv3 = _f32(inputs["bn3_g"]) / np.sqrt(_f32(inputs["bn3_v"]) + EPS)
    sh3c = (_f32(inputs["fc1_b"]) - _f32(inputs["bn3_m"])) * inv3 \
        + _f32(inputs["bn3_b"])
    cst[:, 4:8] = inv3.reshape(4, 128).T
    cst[:, 8:12] = sh3c.reshape(4, 128).T
    inv4 = _f32(inputs["bn4_g"]) / np.sqrt(_f32(inputs["bn4_v"]) + EPS)
    sh4c = (_f32(inputs["fc2_b"]) - _f32(inputs["bn4_m"])) * inv4 \
        + _f32(inputs["bn4_b"])
    cst[:, 12:14] = inv4.reshape(2, 128).T
    cst[:, 14:16] = sh4c.reshape(2, 128).T
    cst[:, 16:26] = _f32(inputs["fc3_b"])[None, :]

    # ---- fc1+fc2+w2 bit-packed into one [128, 2200] u8 tensor ----
    # fc1: kt = yx//2, kp = c + 64*(yx%2), bits along m (cols 0:2048);
    # fc2 at 2048:2176; w2 (rows 0:96 only) at 2176:2200
    fw1 = np.sign(_f32(inputs["fc1_w"]))                 # [512,4096]
    A = fw1.T.reshape(64, 64, 512)                       # [c][yx][m]
    Bm = A.reshape(64, 32, 2, 512)                       # [c][kt][yxp][m]
    fc1_st = np.ascontiguousarray(
        Bm.transpose(1, 2, 0, 3).reshape(32, 128, 512))  # [kt][kp][m]
    fw2 = np.sign(_f32(inputs["fc2_w"]))                 # [256,512]
    fc2_st = np.ascontiguousarray(fw2.T.reshape(4, 128, 256))
    w2v = np.zeros((128, 192), np.float32)               # [p][dy*64+m]
    w2v[:96] = np.ascontiguousarray(
        w2_st.transpose(1, 0, 2)).reshape(96, 192)

    wpk = np.zeros((128, 2200), np.uint8)
    for st, off in ((fc1_st.transpose(1, 0, 2).reshape(128, -1), 0),
                    (fc2_st.transpose(1, 0, 2).reshape(128, -1), 2048),
                    (w2v, 2176)):
        bits = (st > 0).astype(np.uint8)
        for b in range(8):
            wpk[:, off:off + st.shape[1] // 8] |= (bits[:, b::8]
                                                   << np.uint8(b))

    # cst ships as a bf16 hi/lo pair alongside wbf (one bf16 arg)
    ch = cst.astype(BF)
    cl = (cst - ch.astype(np.float32)).astype(BF)
    sbf = np.concatenate([wbf, ch, cl], axis=1)          # [128, 232]

    # weights ship 8-way column-sharded and AllGather on device; x ships
    # split into XSPLIT args (a few parallel tunnel streams beat one)
    PER = NCHUNK // XSPLIT
    in_maps = []
    for ci in range(NCORES):
        m = {
            "s8": np.ascontiguousarray(wpk[:, 275 * ci:275 * (ci + 1)]),
            "sbf": np.ascontiguousarray(sbf[:, 29 * ci:29 * (ci + 1)]),
        }
        for k in range(XSPLIT):
            m[f"xq{k}"] = np.ascontiguousarray(xq[ci, k * PER:(k + 1) * PER])
        in_maps.append(m)
    return in_maps


def _build_module(reps=1):
    import concourse.bass as bass
    import concourse.mybir as mybir
    import concourse.tile as tile
    from concourse import bacc
    from contextlib import ExitStack

    F32 = mybir.dt.float32
    BF16 = mybir.dt.bfloat16
    FP8 = mybir.dt.float8e4
    U8 = mybir.dt.uint8
    AF = mybir.ActivationFunctionType
    ALU = mybir.AluOpType

    nc = bacc.Bacc("TRN2", target_bir_lowering=False, debug=False,
                   num_devices=NCORES)

    U16 = mybir.dt.uint16
    RG = [list(range(NCORES))]

    # ---- DRAM tensors: weights arrive 8-way sharded, AllGather on-device;
    # x is one u16 tensor per core: [hi16 | nibble-pairs] per chunk row ----
    XROW = XCHW + XCHM // 2
    d_xq = [nc.dram_tensor(f"xq{k}", [NCHUNK // XSPLIT, XROW], U16,
                           kind="ExternalInput") for k in range(XSPLIT)]
    shards = {}
    for nm, shp, dt in (("s8", [128, 275], U8), ("sbf", [128, 29], BF16)):
        d_s = nc.dram_tensor(nm, shp, dt, kind="ExternalInput")
        d_i = nc.dram_tensor("i" + nm, shp, dt, kind="Internal")
        d_g = nc.dram_tensor("g" + nm, [NCORES] + shp, dt, kind="Internal",
                             addr_space="Shared")
        shards[nm] = (d_s, d_i, d_g)
    d_out = nc.dram_tensor("out", [BC, 10], F32, kind="ExternalOutput")

    NBUF = 3                       # sp/ic2 ping-pong depth
    NXB = 2                        # ic1 staging ping-pong depth
    SPW = 4 * CH * 18              # sp payload per partition
    SP_FREE = SPW + 8
    IC2_FREE = 18 * CH * 18 + 8    # (R 18, s CH, W 18)

    with tile.TileContext(nc) as tc, ExitStack() as ctx:
        const = ctx.enter_context(tc.tile_pool(name="const", bufs=1))
        wk = ctx.enter_context(tc.tile_pool(name="wk", bufs=3))
        pp1 = ctx.enter_context(tc.tile_pool(name="pp1", bufs=3, space="PSUM"))
        pp2 = ctx.enter_context(tc.tile_pool(name="pp2", bufs=2, space="PSUM"))

        # ---- persistent tiles ----
        sbf_sb = const.tile([128, 232], BF16, tag="sbf")
        w2_sb = const.tile([96, 3, 64], FP8, tag="w2")
        fc1w_sb = const.tile([128, 32, 512], FP8, tag="fc1w")
        fc2w_sb = const.tile([128, 4, 256], FP8, tag="fc2w")
        cst_sb = const.tile([128, 32], F32, tag="cst")
        pk_sb = const.tile([128, 2200], U8, tag="pk")
        tmp_sb = const.tile([128, 2200], U8, tag="tmp")
        qh_t = [const.tile([128, 4096], U16, tag=f"qh{i}", name=f"qh{i}")
                for i in range(NXB)]
        qm_t = [const.tile([128, 1024], U16, tag=f"qm{i}", name=f"qm{i}")
                for i in range(NXB)]
        qmu_sb = const.tile([128, 4096], U16, tag="qmu")
        ich_t = [const.tile([128, 4096], BF16, tag=f"ich{i}", name=f"ich{i}")
                 for i in range(NXB)]
        icl_t = [const.tile([128, 4096], BF16, tag=f"icl{i}", name=f"icl{i}")
                 for i in range(NXB)]
        tf_sb = const.tile([128, 4096], F32, tag="tf")
        tm_sb = const.tile([128, 4096], F32, tag="tm")
        sp_t = [const.tile([128, SP_FREE], FP8, tag=f"sp{i}", name=f"sp{i}")
                for i in range(NBUF)]
        ic2_t = [const.tile([96, IC2_FREE], FP8, tag=f"ic2_{i}", name=f"ic2_{i}")
                 for i in range(NBUF)]
        # s2all: partition (g2, c64), free = yx*128 + sh,
        #   sh = (2*chk + tau)*4 + sl,  sample s = 8*(sh//4) + 4g + sh%4
        s2all = const.tile([128, 64 * 128], FP8, tag="s2all")
        # fc1in: partition kp = c + 64*(yx%2), free = kt*BC + (g*128 + sh)
        fc1in = const.tile([128, 32 * BC], FP8, tag="fc1in")
        s3_sb = const.tile([128, 4, BC], FP8, tag="s3")
        u4_sb = const.tile([128, 2, BC], F32, tag="u4")
        s4h_sb = const.tile([128, 2, BC], BF16, tag="s4h")
        s4l_sb = const.tile([128, 2, BC], BF16, tag="s4l")
        s4r_sb = const.tile([128, 2, BC], F32, tag="s4r")

        w1v = sbf_sb[:, 0:128]

        def ap_of(t, dims, off=0):
            return bass.AP(tensor=t.tensor, offset=t.offset + off,
                           ap=[list(t.ap[0])] + [list(d) for d in dims])

        # ---- setup: AllGather the weight shards, then load to SBUF ----
        for nm, (d_s, d_i, d_g) in shards.items():
            nc.gpsimd.dma_start(d_i.ap(), d_s.ap())
            nc.gpsimd.collective_compute(
                "AllGather", ALU.bypass, RG, ins=[d_i.ap()], outs=[d_g.ap()])
        for c in range(NCORES):
            g = shards["s8"][2].ap()[c]
            nc.gpsimd.dma_start(pk_sb[:, 275 * c:275 * (c + 1)], g)
            g = shards["sbf"][2].ap()[c]
            nc.gpsimd.dma_start(sbf_sb[:, 29 * c:29 * (c + 1)], g)
        # cst = bf16 hi + bf16 lo
        nc.vector.tensor_add(cst_sb[:], sbf_sb[:, 168:200],
                             sbf_sb[:, 200:232])

        # fc1w/fc2w/w2 unpack: bit b of packed byte mb -> m = 8*mb + b
        for b in range(8):
            nc.vector.tensor_scalar(tmp_sb[:], pk_sb[:], b, 1,
                                    ALU.logical_shift_right, ALU.bitwise_and)
            dst = bass.AP(tensor=fc1w_sb.tensor, offset=fc1w_sb.offset + b,
                          ap=[list(fc1w_sb.ap[0]), [512, 32], [8, 64]])
            nc.vector.tensor_scalar(dst, bass.AP(
                tensor=tmp_sb.tensor, offset=tmp_sb.offset,
                ap=[list(tmp_sb.ap[0]), [1, 2048]]), 2.0, -1.0,
                ALU.mult, ALU.add)
            dst2 = bass.AP(tensor=fc2w_sb.tensor, offset=fc2w_sb.offset + b,
                           ap=[list(fc2w_sb.ap[0]), [256, 4], [8, 32]])
            nc.vector.tensor_scalar(dst2, bass.AP(
                tensor=tmp_sb.tensor, offset=tmp_sb.offset + 2048,
                ap=[list(tmp_sb.ap[0]), [1, 128]]), 2.0, -1.0,
                ALU.mult, ALU.add)
            dst3 = bass.AP(tensor=w2_sb.tensor, offset=w2_sb.offset + b,
                           ap=[list(w2_sb.ap[0]), [8, 24]])
            tv = tmp_sb[0:96]
            nc.vector.tensor_scalar(dst3, bass.AP(
                tensor=tv.tensor, offset=tv.offset + 2176,
                ap=[list(tv.ap[0]), [1, 24]]), 2.0, -1.0,
                ALU.mult, ALU.add)

        # init the gather tiles to the encoding of 0.0 once: pad partitions
        # and clipped row slabs stay at q0 forever and decode to exact 0.
        for t in qh_t:
            nc.vector.memset(t[:], Q0 >> 4)
        for t in qm_t:
            nc.vector.memset(t[:], 0)
        # sp pads: cols w==0 and w==17 of each 18-block, plus the 8 slack cols
        for t in sp_t:
            nc.vector.memset(
                ap_of(t, [[18, SPW // 18], [17, 2], [1, 1]]), 0.0)
            nc.vector.memset(ap_of(t, [[1, 8]], SPW), 0.0)
        # ic2 pads: halo rows R=0 and R=17 (+ slack)
        for t in ic2_t:
            nc.vector.memset(
                ap_of(t, [[17 * CH * 18, 2], [1, CH * 18]]), 0.0)
            nc.vector.memset(ap_of(t, [[1, 8]], 18 * CH * 18), 0.0)

        dma_engs = [nc.sync, nc.scalar, nc.gpsimd]

        for _rep in range(reps):
            # ================= chunk loop (software-pipelined) =================
            # iteration k emits conv1 of chunk k, then conv2 of chunk k-1, so
            # the in-order PE queue never stalls on chunk k-1's ic2 DMAs.
            def conv1_part(chk):
                sp = sp_t[chk % NBUF]
                ic2 = ic2_t[chk % NBUF]
                qh = qh_t[chk % NXB]
                qm = qm_t[chk % NXB]
                ich = ich_t[chk % NXB]
                icl = icl_t[chk % NXB]

                # ---- on-device im2col: 36 gather DMAs per half ----
                # dst w-range / src col-range clip at the image edge; the
                # unwritten halo positions keep their q0 memset (decode 0).
                qi = 0
                for tgt, base, G in ((qh, 0, 16), (qm, XCHW, 4)):
                    for j in range(4):
                        for dy in range(3):
                            r0, nr = 0, 8
                            if j == 0 and dy == 0:
                                r0, nr = 1, 7
                            if j == 3 and dy == 2:
                                r0, nr = 0, 7
                            PER = NCHUNK // XSPLIT
                            dxt = d_xq[chk // PER]
                            soff = (chk % PER) * XROW + base \
                                + (8 * j + dy + r0 - 1) * (32 * G)
                            for dx in range(3):
                                p0 = 32 * j + 9 * dy + 3 * dx
                                run = 32 * G if dx == 1 else 31 * G
                                pv = tgt[p0:p0 + 3]
                                dst = bass.AP(
                                    tensor=pv.tensor,
                                    offset=pv.offset + r0 * 32 * G
                                    + (G if dx == 0 else 0),
                                    ap=[list(pv.ap[0]), [32 * G, nr],
                                        [1, run]])
                                src = bass.AP(
                                    tensor=dxt,
                                    offset=soff + (G if dx == 2 else 0),
                                    ap=[[1024 * G, 3], [32 * G, nr],
                                        [1, run]])
                                dma_engs[qi % 3].dma_start(dst, src)
                                qi += 1

                # ---- nibble unpack + int20 decode + bf16 hi/lo split ----
                for n in range(4):
                    nc.vector.tensor_scalar(
                        ap_of(qmu_sb, [[16, 256], [4, 4]], n), qm[:],
                        4 * n, 15, ALU.logical_shift_right, ALU.bitwise_and)
                nc.vector.tensor_scalar(tf_sb[:], qh[:], C_HI, None, ALU.mult)
                nc.vector.tensor_scalar(tm_sb[:], qmu_sb[:], C_LO, -6.0,
                                        ALU.mult, ALU.add)
                nc.vector.tensor_add(tf_sb[:], tf_sb[:], tm_sb[:])
                nc.vector.tensor_copy(ich[:], tf_sb[:])
                nc.vector.tensor_sub(tm_sb[:], tf_sb[:], ich[:])
                nc.vector.tensor_copy(icl[:], tm_sb[:])

                # ---- conv1: 4 psum tiles; tile tau = pooled row pair ----
                # psum free = dr*512 + w*16 + s
                for tau in range(4):
                    p1 = pp1.tile([128, 1024], F32, tag="c1")
                    for dr in range(2):
                        sl = bass.ds((2 * tau + dr) * 512, 512)
                        nc.tensor.matmul(p1[:, dr * 512:(dr + 1) * 512],
                                         w1v, ich[:, sl],
                                         start=True, stop=False)
                        nc.tensor.matmul(p1[:, dr * 512:(dr + 1) * 512],
                                         w1v, icl[:, sl],
                                         start=False, stop=True)
                    # DVE: 2x2 maxpool in ONE XY-window reduce from psum
                    # out free = 16*s + wp
                    pl1 = wk.tile([128, 256], F32, tag="pl1")
                    nc.vector.tensor_reduce(
                        ap_of(pl1, [[16, 16], [1, 16]]),
                        ap_of(p1, [[1, 16], [32, 16], [512, 2], [16, 2]]),
                        mybir.AxisListType.XY, ALU.max)
                    # ACT: sign(bn1) -> +-1 fp8 straight into sp
                    # sp free = rr*288 + s*18 + (wp+1), rr = tau
                    nc.scalar.activation(
                        ap_of(sp, [[18, 16], [1, 16]], tau * 288 + 1),
                        pl1[:], AF.Sign,
                        bias=cst_sb[:, 1:2], scale=cst_sb[:, 0:1])

                # ---- ic2 build: 3 contiguous-run DMAs (SP, ACT, Pool) ----
                RUN = 4 * CH * 18
                for dx in range(3):
                    src = bass.AP(tensor=sp.tensor, offset=sp.offset + dx,
                                  ap=[list(sp.ap[0]), [1, RUN]])
                    dst_t = ic2[32 * dx:32 * (dx + 1)]
                    dst = bass.AP(tensor=dst_t.tensor,
                                  offset=dst_t.offset + CH * 18,
                                  ap=[list(dst_t.ap[0]), [RUN, 4], [1, RUN]])
                    eng = (nc.sync, nc.scalar, nc.gpsimd)[dx]
                    eng.dma_start(dst, src)

            def conv2_part(chk):
                ic2 = ic2_t[chk % NBUF]
                # ---- conv2: 4 one-bank col-packed psum tiles (tau, h) ----
                for tau in range(2):
                    for h in range(2):
                        p2 = pp2.tile([128, 512], F32, tag="c2")
                        for g in range(2):
                            tp = (0, 64 * g) if g else None
                            for dy in range(3):
                                s0 = 8 * tau + 4 * g + 2 * h
                                mv = bass.AP(
                                    tensor=ic2.tensor,
                                    offset=ic2.offset + s0 * 18
                                    + dy * (CH * 18),
                                    ap=[list(ic2.ap[0]), [18, 2],
                                        [CH * 18, 16], [1, 16]])
                                nc.tensor.matmul(
                                    p2[64 * g:64 * (g + 1), :],
                                    w2_sb[:, dy, :], mv,
                                    start=(dy == 0), stop=(dy == 2),
                                    tile_position=tp)
                        # DVE: 2x2 maxpool, one XY-window reduce
                        xm2b = wk.tile([128, 128], F32, tag="xm2b")
                        nc.vector.tensor_reduce(
                            ap_of(xm2b, [[8, 16], [1, 8]]),
                            ap_of(p2, [[32, 16], [2, 8], [16, 2], [1, 2]]),
                            mybir.AxisListType.XY, ALU.max)
                        # ACT sign(bn2) -> +-1 fp8 into s2all
                        sh0 = (2 * chk + tau) * 4 + 2 * h
                        nc.scalar.activation(
                            ap_of(s2all, [[1, 2], [1024, 8], [128, 8]], sh0),
                            xm2b[:], AF.Sign,
                            bias=cst_sb[:, 3:4], scale=cst_sb[:, 2:3])

                # ---- repack waves: one DMA per (g, yxp, wave) ----
                # waves at chk 7 (sh 0:64), 11 (64:96), 15 (96:128) so the
                # final fc1 dependency is only a quarter-size transfer
                WAVES = {7: (0, 64), 11: (64, 32), 15: (96, 32)}
                if chk in WAVES:
                    lo, ln = WAVES[chk]
                    for g in range(2):
                        for yxp in range(2):
                            src_t = s2all[64 * g:64 * (g + 1)]
                            src = bass.AP(
                                tensor=src_t.tensor,
                                offset=src_t.offset + 128 * yxp + lo,
                                ap=[list(src_t.ap[0]), [256, 32], [1, ln]])
                            dst_t = fc1in[64 * yxp:64 * (yxp + 1)]
                            dst = bass.AP(
                                tensor=dst_t.tensor,
                                offset=dst_t.offset + g * 128 + lo,
                                ap=[list(dst_t.ap[0]), [256, 32], [1, ln]])
                            eng = (nc.sync, nc.scalar, nc.gpsimd,
                                   nc.gpsimd)[2 * g + yxp]
                            eng.dma_start(dst, src)

            conv1_part(0)
            for chk in range(1, NCHUNK):
                conv1_part(chk)
                conv2_part(chk - 1)
            conv2_part(NCHUNK - 1)

            # ================= fc phase =================
            # fc1: 4 m-tiles, fp8 DoubleRow over kt pairs, column-sliced:
            # cols filled by waves 1+2 (sh 0:96 of both g-blocks) run while
            # wave 3 is still transferring; the wave-3 cols follow.
            pfs = []
            for m in range(4):
                pf = (pp1 if m < 3 else pp2).tile(
                    [128, 1024] if m < 3 else [128, 512], F32,
                    tag="c1" if m < 3 else "c2", name=f"pf_{m}")
                pfs.append(pf)
            SLICES = [[(0, 96), (128, 96)], [(96, 32), (224, 32)]]
            for phase in range(2):
                for m in range(4):
                    pf = pfs[m]
                    for c0, wdt in SLICES[phase]:
                        for kt in range(16):
                            lw = bass.AP(
                                tensor=fc1w_sb.tensor,
                                offset=fc1w_sb.offset + 2 * kt * 512
                                + 128 * m,
                                ap=[list(fc1w_sb.ap[0]), [512, 2], [1, 128]])
                            mv = bass.AP(
                                tensor=fc1in.tensor,
                                offset=fc1in.offset + 2 * kt * BC + c0,
                                ap=[list(fc1in.ap[0]), [BC, 2], [1, wdt]])
                            nc.tensor.matmul(
                                pf[:, c0:c0 + wdt], lw, mv,
                                start=(kt == 0), stop=(kt == 15),
                                perf_mode=mybir.MatmulPerfMode.DoubleRow)
            for m in range(4):
                nc.scalar.activation(s3_sb[:, m, :], pfs[m][:, :BC], AF.Sign,
                                     bias=cst_sb[:, 8 + m:9 + m],
                                     scale=cst_sb[:, 4 + m:5 + m])

            # fc2: 2 m-tiles, fp8 DoubleRow over kt pairs; clip + hi/lo
            # split per m2-half so the DVE chain overlaps fc2's second half
            for m2 in range(2):
                pg = pp1.tile([128, 1024], F32, tag="c1")
                for kt in range(2):
                    lw = bass.AP(
                        tensor=fc2w_sb.tensor,
                        offset=fc2w_sb.offset + 2 * kt * 256 + 128 * m2,
                        ap=[list(fc2w_sb.ap[0]), [256, 2], [1, 128]])
                    mv = bass.AP(
                        tensor=s3_sb.tensor,
                        offset=s3_sb.offset + 2 * kt * BC,
                        ap=[list(s3_sb.ap[0]), [BC, 2], [1, BC]])
                    nc.tensor.matmul(pg[:, :BC], lw, mv,
                                     start=(kt == 0), stop=(kt == 1),
                                     perf_mode=mybir.MatmulPerfMode.DoubleRow)
                nc.scalar.activation(u4_sb[:, m2, :], pg[:, :BC], AF.Identity,
                                     bias=cst_sb[:, 14 + m2:15 + m2],
                                     scale=cst_sb[:, 12 + m2:13 + m2])
                nc.vector.tensor_scalar(u4_sb[:, m2, :], u4_sb[:, m2, :],
                                        1.0, -1.0, ALU.min, ALU.max)
                nc.vector.tensor_copy(s4h_sb[:, m2, :], u4_sb[:, m2, :])
                nc.vector.tensor_sub(s4r_sb[:, m2, :], u4_sb[:, m2, :],
                                     s4h_sb[:, m2, :])
                nc.vector.tensor_copy(s4l_sb[:, m2, :], s4r_sb[:, m2, :])

            # fc3 + log_softmax; batch tile bt == g block of fc1 columns.
            h3s, mxs, negs, ses, lss = [], [], [], [], []
            for bt in range(2):
                ph = pp2.tile([128, 512], F32, tag="c2")
                mms = []
                for kt in range(2):
                    lh = s4h_sb[:, kt, 128 * bt:128 * (bt + 1)]
                    ll = s4l_sb[:, kt, 128 * bt:128 * (bt + 1)]
                    w3hv = sbf_sb[:, 128 + 10 * kt:138 + 10 * kt]
                    w3lv = sbf_sb[:, 148 + 10 * kt:158 + 10 * kt]
                    mms += [(lh, w3hv), (ll, w3hv), (lh, w3lv)]
                for i, (lhs, rhs) in enumerate(mms):
                    nc.tensor.matmul(ph[:, :10], lhs, rhs,
                                     start=(i == 0), stop=(i == len(mms) - 1))
                h3 = wk.tile([128, 10], F32, tag="h3", name=f"h3_{bt}")
                nc.vector.tensor_add(h3[:], ph[:, :10], cst_sb[:, 16:26])
                mx = wk.tile([128, 1], F32, tag="mx", name=f"mx_{bt}")
                nc.vector.tensor_reduce(mx[:], h3[:], mybir.AxisListType.X,
                                        ALU.max)
                negmx = wk.tile([128, 1], F32, tag="negmx", name=f"negmx_{bt}")
                nc.vector.tensor_scalar_mul(negmx[:], mx[:], -1.0)
                h3s.append(h3)
                mxs.append(mx)
                negs.append(negmx)
            for bt in range(2):     # both Exps together (one act table set)
                et = wk.tile([128, 10], F32, tag="et", name=f"et_{bt}")
                se = wk.tile([128, 1], F32, tag="se", name=f"se_{bt}")
                nc.scalar.activation(et[:], h3s[bt][:], AF.Exp,
                                     bias=negs[bt][:], scale=1.0,
                                     accum_out=se[:])
                ses.append(se)
            for bt in range(2):     # then both Lns (single table reload)
                ls = wk.tile([128, 1], F32, tag="ls", name=f"ls_{bt}")
                nc.scalar.activation(ls[:], ses[bt][:], AF.Ln)
                lss.append(ls)
            for bt in range(2):
                tt = wk.tile([128, 1], F32, tag="tt", name=f"tt_{bt}")
                nc.vector.tensor_add(tt[:], mxs[bt][:], lss[bt][:])
                o = wk.tile([128, 10], F32, tag="o", name=f"o_{bt}")
                nc.vector.tensor_scalar_sub(o[:], h3s[bt][:], tt[:])
                # un-permute: psum partition p -> sample row
                # s = 8*(p//4) + 4*bt + p%4
                dst = bass.AP(tensor=d_out, offset=bt * 40,
                              ap=[[80, 32], [10, 4], [1, 10]])
                (nc.sync if bt == 0 else nc.scalar).dma_start(dst, o[:])

    nc.compile()
    return nc


def _get_module():
    if "nc" not in _nc_cache:
        nc = _build_module()
        # the module is frozen after compile; memoize its (identical)
        # serialization so per-call re-jits don't re-serialize ~30ms worth
        js = nc.to_json_bytes()
        nc.to_json_bytes = lambda: js
        _nc_cache["nc"] = nc
    return _nc_cache["nc"]


def kernel(**inputs):
    from concourse.bass_utils import run_bass_kernel_spmd

    in_maps = _host_prep(inputs)
    nc = _get_module()
    res = run_bass_kernel_spmd(nc, in_maps, core_ids=list(range(NCORES)))
    out = np.concatenate([r["out"] for r in res.results], axis=0)
    return out.astype(np.float32)
